# revision 34
# baseline (speedup 1.0000x reference)
"""Trainium2 Bass kernel for NodeCentricNewReasoner GNN (3-layer message passing).

Strategy: shard edges by dst-node range across 8 cores (6250 nodes/core).
Each layer: AllGather node features (bf16) to an HBM table -> dma_gather rows
per edge -> segment-sum via one-hot matmuls accumulated in PSUM (f32) ->
node update (GraphConv + LayerNorm + ReLU + residual, f32) on own nodes.
Decoder: SBUF-source transposed dma_gather of final x for src/dst, fully
matmul-based MLP (no per-edge vector ops).
"""

import numpy as np

N = 50000
E = 800000
H = 128
L = 3
C = 8                      # cores
NPC = N // C               # 6250 own nodes per core
BLK = 64                   # nodes per scatter block (one-hot matmul width)
NPADC = 6272               # own nodes padded (98 * 64)
NBLK = NPADC // BLK        # 98
RT = C * NPADC             # 50176 table rows
SPLIT = 32768              # int16 index limit split
EPS = 1e-5

GATHER_CHUNK = 4096        # slots per dma_gather call (layer phase)
GATHER_CHUNK_DEC = 2048    # slots per dma_gather call (decoder phase)
DEC_CHUNK = 512            # decoder psum chunk (slots)
BF16 = True                # bf16 tables/gathers/matmul inputs (f32 accumulate)
NQ = 4                     # swdge queues round-robin


def _wrap_idx(vals):
    """int32 [n] -> wrapped int16 [128, n//16] (16-part wrap, replicated x8)."""
    n = len(vals)
    assert n % 16 == 0
    w = vals.astype(np.int16).reshape(n // 16, 16).T  # [16, n//16]
    return np.ascontiguousarray(np.tile(w, (8, 1)))


def _dt_tab():
    if BF16:
        return np.dtype(np.float16)
    return np.dtype(np.float32)


def _prep(inputs):
    """Host-side preprocessing: sharding, slot layout, weights packing."""
    f32 = np.float32
    src = np.asarray(inputs["edge_index_new"][0]).astype(np.int64)
    dst = np.asarray(inputs["edge_index_new"][1]).astype(np.int64)
    af = np.asarray(inputs["aligned_features"]).astype(f32)
    h_old = np.asarray(inputs["h_nodes_old"]).astype(f32)

    Wf = np.asarray(inputs["Wf"]).astype(f32)
    bf = np.asarray(inputs["bf"]).astype(f32)
    We1 = np.asarray(inputs["We1"]).astype(f32)
    be1 = np.asarray(inputs["be1"]).astype(f32)
    We2 = np.asarray(inputs["We2"]).astype(f32)
    be2 = np.asarray(inputs["be2"]).astype(f32)
    Wrel = np.asarray(inputs["Wrel"]).astype(f32)
    brel = np.asarray(inputs["brel"]).astype(f32)
    Wroot = np.asarray(inputs["Wroot"]).astype(f32)
    gamma = np.asarray(inputs["gamma"]).astype(f32)
    beta = np.asarray(inputs["beta"]).astype(f32)
    Wd1 = np.asarray(inputs["Wd1"]).astype(f32)
    bd1 = np.asarray(inputs["bd1"]).astype(f32)
    Wd2 = np.asarray(inputs["Wd2"]).astype(f32)
    bd2 = np.asarray(inputs["bd2"]).astype(f32)

    core = dst // NPC
    loc = (dst - core * NPC).astype(np.int64)          # local dst in [0, 6250)
    blk = loc // BLK
    col = loc % BLK
    sr = ((src // NPC) * NPADC + (src % NPC)).astype(np.int64)  # src table row
    half = (sr >= SPLIT).astype(np.int64)
    assert NPC < SPLIT  # local dst indices fit the int16 gather index format

    # ---- layer slot layout: group by (src_half, dst_block), budgets = max over cores
    NG = 2 * NBLK
    key = half * NBLK + blk                             # [E]
    cnt = np.zeros((C, NG), np.int64)
    for c in range(C):
        cnt[c] = np.bincount(key[core == c], minlength=NG)
    B = np.maximum(128, ((cnt.max(axis=0) + 127) // 128) * 128)  # [NG] slots per group
    offs = np.zeros(NG + 1, np.int64)
    offs[1:] = np.cumsum(B)
    S_lo = int(B[:NBLK].sum())
    S_hi = int(B[NBLK:].sum())
    Epad = S_lo + S_hi
    T = Epad // 128

    # per-tile static structure
    tile_group = np.repeat(np.arange(NG), B // 128)     # [T]
    tile_block = tile_group % NBLK
    tg = tile_group
    t_start = np.zeros(T, bool)
    t_stop = np.zeros(T, bool)
    t_start[0] = True
    t_start[1:] = tg[1:] != tg[:-1]
    t_stop[-1] = True
    t_stop[:-1] = tg[1:] != tg[:-1]
    grp_is_hi = np.arange(NG) >= NBLK

    # ---- per-core slot data
    per_core = []
    for c in range(C):
        m = core == c
        k_c = key[m]
        order = np.argsort(k_c, kind="stable")
        ksort = k_c[order]
        grp_first = np.searchsorted(ksort, np.arange(NG), side="left")
        pos_sorted = np.arange(len(ksort)) - grp_first[ksort]
        slotpos = offs[ksort] + pos_sorted
        eids = np.nonzero(m)[0][order]

        slot_sr = np.zeros(Epad, np.int64)
        slot_sr[S_lo:] = SPLIT                          # hi-region padding default
        slot_dc = np.full(Epad, 127.0, f32)             # 127 => one-hot always 0
        slot_loc = np.zeros(Epad, np.int64)             # local dst row (V gather)
        slot_af = np.zeros((Epad, 8), f32)
        slot_sr[slotpos] = sr[eids]
        slot_dc[slotpos] = col[eids].astype(f32)
        slot_loc[slotpos] = loc[eids]
        slot_af[slotpos] = af[eids]

        inv = np.full(Epad, -1, np.int64)
        inv[slotpos] = eids

        idx_w = np.concatenate([slot_sr[:S_lo], slot_sr[S_lo:] - SPLIT])
        ho = h_old[c * NPC:(c + 1) * NPC]
        hoT = np.zeros((H, NPADC), f32)
        hoT[:, :NPC] = ho.T
        per_core.append(dict(
            idx_layer=_wrap_idx(idx_w),
            dstc=np.ascontiguousarray(slot_dc.reshape(T, 128).T),         # [128, T]
            dstc64=np.ascontiguousarray(np.broadcast_to(
                slot_dc[None, :], (BLK, Epad)).astype(_dt_tab())),         # [64, Epad]
            afT=np.ascontiguousarray(np.vstack(
                [slot_af.T, np.ones((1, Epad), f32)]).astype(_dt_tab())),  # [9, Epad]
            inv=inv,
            h_oldT=hoT,
        ))

    # ---- packed weights (shared across cores)
    c0 = bf + Wf[:H].sum(axis=0)
    consts = dict(
        Wf2=np.ascontiguousarray(Wf[H:]),                 # rhs [128,128]
        c0_row=np.tile(c0[None, :], (128, 1)),
        We1aug=np.vstack([We1, be1[None, :]]),            # [9,128]
        we2row=np.tile(We2[:, 0][None, :], (128, 1)),
        Wrel0=Wrel[0], Wrel1=Wrel[1], Wrel2=Wrel[2],
        Wroot0=Wroot[0], Wroot1=Wroot[1], Wroot2=Wroot[2],
        brel0=np.tile(brel[0][None, :], (128, 1)),
        brel1=np.tile(brel[1][None, :], (128, 1)),
        brel2=np.tile(brel[2][None, :], (128, 1)),
        gam0=np.tile(gamma[0][None, :], (128, 1)),
        gam1=np.tile(gamma[1][None, :], (128, 1)),
        gam2=np.tile(gamma[2][None, :], (128, 1)),
        bet0=np.tile(beta[0][None, :], (128, 1)),
        bet1=np.tile(beta[1][None, :], (128, 1)),
        bet2=np.tile(beta[2][None, :], (128, 1)),
        Wd1a=np.ascontiguousarray(Wd1[:H]),
        Wd1b=np.ascontiguousarray(Wd1[H:2 * H]),
        Wd1caug=np.vstack([Wd1[2 * H:], bd1[None, :]]),   # [9,128]
        wd2col=np.ascontiguousarray(Wd2[:, 0:1]),         # [128,1] Wd2 along K
        iota64=np.tile(np.arange(BLK, dtype=f32)[None, :], (128, 1)),
        iotacol=np.arange(BLK, dtype=f32).reshape(BLK, 1),
        ident16=np.eye(128, dtype=f32),
        epscol=np.full((128, 1), EPS, f32),
        be2col=np.full((128, 1), be2[0], f32),
        ident=np.eye(128, dtype=f32),
    )
    consts = {k: np.ascontiguousarray(v.astype(f32)) for k, v in consts.items()}
    for k in ("We1aug", "we2row", "Wd1a", "Wd1b", "Wd1caug", "wd2col", "ident16"):
        consts[k] = np.ascontiguousarray(consts[k].astype(_dt_tab()))

    meta = dict(
        B=B, S_lo=S_lo, S_hi=S_hi, Epad=Epad, T=T,
        tile_block=tile_block, tile_group=tile_group,
        t_start=t_start, t_stop=t_stop, grp_is_hi=grp_is_hi,
    )
    return meta, consts, per_core, bd2


def _chunks(total, start_slot, sz=None):
    sz = sz or GATHER_CHUNK
    out = []
    o = 0
    while o < total:
        n = min(sz, total - o)
        out.append((start_slot + o, n))
        o += n
    return out


def _build(meta, dbg=False, sim1=False, abl=()):
    """abl: ablation flags for timing experiments (default none):
    'rep2'/'rep3' repeat whole computation; 'nogather' skip dma_gathers;
    'nocoll' replace AllGather with local copy; 'nodec' skip decoder."""
    import concourse.bacc as bacc
    import concourse.tile as tile
    import concourse.mybir as mybir
    from concourse import library_config

    f32 = mybir.dt.float32
    i16 = mybir.dt.int16
    dtt = mybir.dt.float16 if BF16 else mybir.dt.float32
    dttn = 2 if BF16 else 4   # bytes
    Alu = mybir.AluOpType
    Act = mybir.ActivationFunctionType

    S_lo, S_hi, Epad, T = meta["S_lo"], meta["S_hi"], meta["Epad"], meta["T"]
    tile_block = meta["tile_block"]
    tile_group = meta["tile_group"]
    t_start = meta["t_start"]
    t_stop = meta["t_stop"]

    # evac plan: group g -> mode at its last tile. 0: copy into aggT, 1: add
    B = meta["B"]
    first_half_of_block = {}
    evac_mode = {}
    for g in range(2 * NBLK):
        b = g % NBLK
        if B[g] == 0:
            continue
        if b not in first_half_of_block:
            first_half_of_block[b] = g
            evac_mode[g] = 0
        else:
            evac_mode[g] = 1

    nc = bacc.Bacc("TRN2", target_bir_lowering=False, debug=False,
                   enable_asserts=False, num_devices=1 if sim1 else C,
                   num_swdge_queues=NQ)

    # SWDGE completion sems are assigned round-robin (mod 8) over ALL SWDGE
    # DMA instructions in program order (tile_sem_assignment.next_sw_dma_idx),
    # and each sem is locked to one queue. queue = global_index % NQ (NQ
    # divides 8) keeps every sem slot on a single queue.
    _gq = [0]

    def next_queue():
        q = _gq[0] % NQ
        _gq[0] += 1
        return q

    def din(name, shape, dt=f32):
        return nc.dram_tensor(name, shape, dt, kind="ExternalInput").ap()

    h_oldT = din("h_oldT", [H, NPADC])
    idx_layer = din("idx_layer", [128, Epad // 16], i16)
    dstc_d = din("dstc", [128, T])
    dstc64_d = din("dstc64", [BLK, Epad], dtt)
    afT_d = din("afT", [9, Epad], dtt)
    cn = {k: din(k, [128, 128]) for k in [
        "Wf2", "c0_row", "Wrel0", "Wrel1", "Wrel2",
        "Wroot0", "Wroot1", "Wroot2", "brel0", "brel1", "brel2",
        "gam0", "gam1", "gam2", "bet0", "bet1", "bet2", "ident"]}
    for k in ("Wd1a", "Wd1b", "we2row", "ident16"):
        cn[k] = din(k, [128, 128], dtt)
    cn["wd2col"] = din("wd2col", [128, 1], dtt)
    cn["We1aug"] = din("We1aug", [9, 128], dtt)
    cn["Wd1caug"] = din("Wd1caug", [9, 128], dtt)
    cn["iota64"] = din("iota64", [128, BLK])
    cn["iotacol"] = din("iotacol", [BLK, 1])
    cn["epscol"] = din("epscol", [128, 1])
    cn["be2col"] = din("be2col", [128, 1])
    out_d = nc.dram_tensor("out", [1, Epad], f32, kind="ExternalOutput").ap()
    dbg_d = {}
    if "dbguv" in abl:
        dbg_d["U"] = nc.dram_tensor("dbg_U", [RT, H], dtt, kind="ExternalOutput").ap()
        dbg_d["V"] = nc.dram_tensor("dbg_V", [NPADC, H], dtt, kind="ExternalOutput").ap()
    if dbg:
        for l in range(L + 1):
            dbg_d[f"x{l}"] = nc.dram_tensor(
                f"dbg_x{l}", [NPADC, H], f32, kind="ExternalOutput").ap()
        dbg_d["ew"] = nc.dram_tensor("dbg_ew", [128, T], f32, kind="ExternalOutput").ap()
        dbg_d["agg0"] = nc.dram_tensor("dbg_agg0", [H, NPADC], f32, kind="ExternalOutput").ap()

    NT = NPADC // 128  # node tiles

    with tile.TileContext(nc) as tc:
        with tc.tile_pool(name="const", bufs=1) as cpool, \
             tc.tile_pool(name="dram", bufs=1, space="DRAM") as dpool:

            nc.gpsimd.load_library(library_config.mlp)

            ct = {}
            for k, d in cn.items():
                dt_ = dtt if k in ("We1aug", "we2row", "Wd1a", "Wd1b",
                                   "Wd1caug", "wd2col", "ident16") else f32
                t_ = cpool.tile(list(d.shape), dt_, tag=f"c_{k}", name=f"c_{k}")
                nc.sync.dma_start(t_[:], d[:])
                ct[k] = t_

            agin = [dpool.tile([NPADC, H], dtt, tag=f"agin{l}", name=f"agin{l}")
                    for l in range(L + 1)]
            tabs = [dpool.tile([RT, H], dtt, tag=f"tab{l}", name=f"tab{l}")
                    for l in range(L + 1)]
            # V = x3 @ Wd1b for own nodes (written at layer-3 export, read by
            # the decoder after the layer pools are freed)
            V_dram = dpool.tile([NPADC, H], dtt, tag="V_dram", name="V_dram")

            REP = 3 if "rep3" in abl else (2 if "rep2" in abl else 1)
            for _rep in range(REP):
                # ======== layer phase pools (freed before decoder) ========
                with tc.tile_pool(name="lpersist", bufs=1) as ppool, \
                     tc.tile_pool(name="work", bufs=2) as wpool, \
                     tc.tile_pool(name="gath", bufs=1) as gpool, \
                     tc.tile_pool(name="psum", bufs=4, space="PSUM") as pspool, \
                     tc.tile_pool(name="psum2", bufs=4, space="PSUM") as ps2pool:

                    dstc = ppool.tile([128, T], f32, tag="dstc")
                    nc.sync.dma_start(dstc[:], dstc_d[:])
                    ew = ppool.tile([128, T], f32, tag="ew")
                    x_own = ppool.tile([128, NT, H], f32, tag="x_own")
                    xT_own = ppool.tile([H, NPADC], f32, tag="xT_own")
                    aggT = ppool.tile([H, NPADC], f32, tag="aggT")

                    def export_tile(l, nt):
                        # l < L: export x_l rows (gather table for layer l).
                        # l == L: export U = x3 @ Wd1a instead (decoder table),
                        # and V = x3 @ Wd1b for own nodes via DRAM.
                        pst = ps2pool.tile([128, 128], f32, tag="ps_a")
                        nc.tensor.transpose(pst[:], x_own[:, nt, :], ct["ident"][:])
                        if l < L:
                            nc.vector.tensor_copy(
                                xT_own[:, nt * 128:(nt + 1) * 128], pst[:])
                            xrow = wpool.tile([128, 128], dtt, tag="w_xrow")
                            nc.scalar.copy(xrow[:], x_own[:, nt, :])
                            nc.sync.dma_start(
                                agin[l][nt * 128:(nt + 1) * 128, :], xrow[:])
                        else:
                            xTb = wpool.tile([128, 128], dtt, tag="w_xTb")
                            nc.scalar.copy(xTb[:], pst[:])
                            psU = ps2pool.tile([128, 128], f32, tag="ps_a")
                            nc.tensor.matmul(psU[:], xTb[:], ct["Wd1a"][:],
                                             start=True, stop=True)
                            urow = wpool.tile([128, 128], dtt, tag="w_xrow")
                            nc.scalar.copy(urow[:], psU[:])
                            nc.sync.dma_start(
                                agin[l][nt * 128:(nt + 1) * 128, :], urow[:])
                            psV = ps2pool.tile([128, 128], f32, tag="ps_a")
                            nc.tensor.matmul(psV[:], xTb[:], ct["Wd1b"][:],
                                             start=True, stop=True)
                            vrow = wpool.tile([128, 128], dtt, tag="w_vrow")
                            nc.scalar.copy(vrow[:], psV[:])
                            nc.sync.dma_start(
                                V_dram[nt * 128:(nt + 1) * 128, :], vrow[:])

                    def export_collective(l):
                        if sim1 or "nocoll" in abl:
                            nc.sync.dma_start(tabs[l][0:NPADC, :], agin[l][:])
                        else:
                            nc.gpsimd.collective_compute(
                                "AllGather", mybir.AluOpType.bypass,
                                ins=[agin[l].opt()], outs=[tabs[l].opt()],
                                replica_groups=[list(range(C))],
                            )

                    def node_transpose_and_export(l):
                        for nt in range(NT):
                            export_tile(l, nt)
                        export_collective(l)

                    # ===== Phase 1: x0 = relu(h_old @ Wf2 + c0)
                    for nt in range(NT):
                        hoT_t = wpool.tile([128, 128], f32, tag="w_hoT")
                        nc.sync.dma_start(hoT_t[:], h_oldT[:, nt * 128:(nt + 1) * 128])
                        ps = ps2pool.tile([128, 128], f32, tag="ps_a")
                        nc.tensor.matmul(ps[:], hoT_t[:], ct["Wf2"][:], start=True, stop=True)
                        tmp = wpool.tile([128, 128], f32, tag="w_init")
                        nc.vector.scalar_tensor_tensor(
                            tmp[:], ps[:], 1.0, ct["c0_row"][:], op0=Alu.mult, op1=Alu.add)
                        nc.vector.tensor_scalar_max(x_own[:, nt, :], tmp[:], 0.0)
                    node_transpose_and_export(0)
                    if dbg:
                        for nt in range(NT):
                            nc.sync.dma_start(dbg_d["x0"][nt * 128:(nt + 1) * 128, :],
                                              x_own[:, nt, :])

                    # ===== Phase 2: edge weights ew
                    AFC = 4096
                    for o in range(0, Epad, AFC):
                        n = min(AFC, Epad - o)
                        aft = wpool.tile([9, AFC], dtt, tag="w_aft")
                        nc.sync.dma_start(aft[:, :n], afT_d[:, o:o + n])
                        for tt in range(n // 128):
                            t0 = o // 128 + tt
                            pse = ps2pool.tile([128, 128], f32, tag="ps_a")
                            nc.tensor.matmul(pse[:], aft[:, tt * 128:(tt + 1) * 128],
                                             ct["We1aug"][:], start=True, stop=True)
                            h1 = wpool.tile([128, 128], dtt, tag="w_h1")
                            nc.scalar.activation(h1[:], pse[:], Act.Relu)
                            scr = wpool.tile([128, 128], f32, tag="w_scr")
                            nc.vector.scalar_tensor_tensor(
                                scr[:], h1[:], 1.0, ct["we2row"][:],
                                op0=Alu.mult, op1=Alu.mult,
                                accum_out=ew[:, t0:t0 + 1])
                    nc.scalar.activation(ew[:], ew[:], Act.Sigmoid, bias=ct["be2col"][:, 0:1])
                    if dbg:
                        nc.sync.dma_start(dbg_d["ew"][:], ew[:])

                    # ===== Phase 3: layers
                    def node_update(l, nt):
                        Wrel_t, Wroot_t = ct[f"Wrel{l}"], ct[f"Wroot{l}"]
                        brel_t, gam_t, bet_t = ct[f"brel{l}"], ct[f"gam{l}"], ct[f"bet{l}"]
                        sl = slice(nt * 128, (nt + 1) * 128)
                        psn = ps2pool.tile([128, 128], f32, tag="ps_a")
                        nc.tensor.matmul(psn[:], aggT[:, sl], Wrel_t[:],
                                         start=True, stop=False)
                        nc.tensor.matmul(psn[:], xT_own[:, sl], Wroot_t[:],
                                         start=False, stop=True)
                        hsb = wpool.tile([128, 128], f32, tag="w_h")
                        nc.vector.scalar_tensor_tensor(
                            hsb[:], psn[:], 1.0, brel_t[:], op0=Alu.mult, op1=Alu.add)
                        mu = wpool.tile([128, 1], f32, tag="w_mu")
                        nc.vector.tensor_reduce(mu[:], hsb[:],
                                                axis=mybir.AxisListType.X, op=Alu.add)
                        nc.scalar.mul(mu[:], mu[:], 1.0 / H)
                        cent = wpool.tile([128, 128], f32, tag="w_cent")
                        nc.vector.tensor_scalar(cent[:], hsb[:], mu[:], None,
                                                op0=Alu.subtract)
                        sq = wpool.tile([128, 128], f32, tag="w_sq")
                        vs = wpool.tile([128, 1], f32, tag="w_vs")
                        nc.scalar.activation(sq[:], cent[:], Act.Square, accum_out=vs[:])
                        std = wpool.tile([128, 1], f32, tag="w_std")
                        nc.scalar.activation(std[:], vs[:], Act.Sqrt,
                                             bias=ct["epscol"][:, 0:1], scale=1.0 / H)
                        rstd = wpool.tile([128, 1], f32, tag="w_rstd")
                        nc.vector.reciprocal(rstd[:], std[:])
                        t1 = wpool.tile([128, 128], f32, tag="w_t1")
                        nc.vector.scalar_tensor_tensor(
                            t1[:], cent[:], rstd[:], gam_t[:], op0=Alu.mult, op1=Alu.mult)
                        t2 = wpool.tile([128, 128], f32, tag="w_t2")
                        nc.vector.tensor_tensor(t2[:], t1[:], bet_t[:], op=Alu.add)
                        nc.vector.scalar_tensor_tensor(
                            x_own[:, nt, :], t2[:], 0.0, x_own[:, nt, :],
                            op0=Alu.max, op1=Alu.add)

                    # group-aligned gather chunks: PSUM accumulation chains
                    # never cross a chunk boundary
                    offs_b = np.zeros(2 * NBLK + 1, np.int64)
                    offs_b[1:] = np.cumsum(B)

                    def group_plan(g_lo, g_hi):
                        chunks = []
                        cur_o, cur_n = None, 0
                        for g in range(g_lo, g_hi):
                            bg = int(B[g])
                            assert bg <= GATHER_CHUNK
                            if cur_o is None:
                                cur_o, cur_n = int(offs_b[g]), 0
                            elif cur_n + bg > GATHER_CHUNK:
                                chunks.append((cur_o, cur_n))
                                cur_o, cur_n = int(offs_b[g]), 0
                            cur_n += bg
                        if cur_n:
                            chunks.append((cur_o, cur_n))
                        return chunks

                    gplan = group_plan(0, NBLK) + group_plan(NBLK, 2 * NBLK)

                    for l in range(L):
                        qi = 0
                        ps_blk = None
                        for (o, n) in gplan:
                            TT = n // 128
                            t0w = o // 128
                            gb = gpool.tile([128, GATHER_CHUNK // 128, H], dtt,
                                            tag="gbuf", bufs=3)
                            iw = gpool.tile([128, GATHER_CHUNK // 16], i16, tag="ibuf", bufs=4)
                            nc.sync.dma_start(iw[:, :n // 16],
                                              idx_layer[:, o // 16:(o + n) // 16])
                            src_ap = tabs[l][0:SPLIT, :] if o < S_lo else tabs[l][SPLIT:RT, :]
                            if "nogather" not in abl:
                                nc.gpsimd.dma_gather(
                                    gb[:, :TT, :], src_ap, iw[:, :n // 16],
                                    num_idxs=n, num_idxs_reg=n, elem_size=H,
                                    queue_num=next_queue(), single_packet=False)
                            else:
                                # same bytes, contiguous: isolates desc-gen cost
                                nc.sync.dma_start(
                                    gb[:, :TT, :],
                                    tabs[l][0:n, :].rearrange(
                                        "(p r) h -> p r h", p=128))
                            qi += 1
                            # one-hot * ew for the whole chunk: 2 wide DVE ops
                            eqw = gpool.tile([128, GATHER_CHUNK // 128, BLK], dtt,
                                             tag="eqw", bufs=2)
                            ohew = gpool.tile([128, GATHER_CHUNK // 128, BLK], dtt,
                                              tag="ohew", bufs=2)
                            dstc_b = dstc[:, t0w:t0w + TT].rearrange(
                                "p (t u) -> p t u", u=1).broadcast_to([128, TT, BLK])
                            ew_b = ew[:, t0w:t0w + TT].rearrange(
                                "p (t u) -> p t u", u=1).broadcast_to([128, TT, BLK])
                            iota_b = ct["iota64"][:].rearrange(
                                "p (u b) -> p u b", u=1).broadcast_to([128, TT, BLK])
                            nc.vector.tensor_tensor(eqw[:, :TT, :], dstc_b, iota_b,
                                                    op=Alu.is_equal)
                            nc.vector.tensor_tensor(ohew[:, :TT, :], eqw[:, :TT, :],
                                                    ew_b, op=Alu.mult)
                            for tt in range(TT):
                                t0 = t0w + tt
                                g = int(tile_group[t0])
                                b = int(tile_block[t0])
                                if t_start[t0]:
                                    ps_blk = pspool.tile([128, BLK], f32, tag="ps_sc")
                                nc.tensor.matmul(ps_blk[:], gb[:, tt, :], ohew[:, tt, :],
                                                 start=bool(t_start[t0]),
                                                 stop=bool(t_stop[t0]))
                                if t_stop[t0]:
                                    sl = aggT[:, b * BLK:(b + 1) * BLK]
                                    if evac_mode[g] == 0:
                                        nc.vector.tensor_copy(sl, ps_blk[:])
                                    else:
                                        nc.vector.tensor_tensor(sl, ps_blk[:], sl, op=Alu.add)
                                    # interleave node updates into the hi-pass:
                                    # pair (2nt, 2nt+1) is final once the hi
                                    # group of block 2nt+1 has been evacuated
                                    if g >= NBLK and b % 2 == 1:
                                        node_update(l, b // 2)
                                        export_tile(l + 1, b // 2)
                        if dbg and l == 0:
                            nc.sync.dma_start(dbg_d["agg0"][:, :], aggT[:, :])
                        export_collective(l + 1)
                        if dbg:
                            for nt in range(NT):
                                nc.sync.dma_start(
                                    dbg_d[f"x{l + 1}"][nt * 128:(nt + 1) * 128, :],
                                    x_own[:, nt, :])
                # lpersist + gath pools freed here

                # ========== Phase 4: decoder (reuses layer slot layout) =========
                # out[e] = wd2 . relu(U[src_e] + V[dst_e] + af_e @ Wd1c + bd1)
                # U[src]: HBM transposed gather from tabs[L] (concurrent-safe
                # across queues, unlike SBUF-source gathers).
                # V[dst]: slots are dst-block-grouped, so V comes from one-hot
                # matmuls against SBUF-resident V_blk; the one-hot [64, n] is
                # built on device from the streamed dstc64 row values.
                with tc.tile_pool(name="dec", bufs=1) as dcp, \
                     tc.tile_pool(name="psdec", bufs=3, space="PSUM") as psd_pool, \
                     tc.tile_pool(name="psout", bufs=4, space="PSUM") as pso_pool:
                    # V per 64-row dst block, base partition 0
                    V_blk = dcp.tile([BLK, NBLK, H], dtt, tag="V_blk")
                    nc.sync.dma_start(
                        V_blk[:, :, :],
                        V_dram[:].rearrange("(b r) h -> r b h", r=BLK))

                    if "dbguv" in abl:
                        nc.sync.dma_start(dbg_d["U"][:, :], tabs[L][:, :])
                        nc.sync.dma_start(dbg_d["V"][:, :], V_dram[:, :])

                    plan = (_chunks(S_lo, 0, GATHER_CHUNK_DEC)
                            + _chunks(S_hi, S_lo, GATHER_CHUNK_DEC))
                    if "nodec" in abl:
                        plan = []
                    for (o, n) in plan:
                        gu = dcp.tile([128, GATHER_CHUNK_DEC // 128, H], dtt,
                                      tag="gu", bufs=3)
                        iu = dcp.tile([128, GATHER_CHUNK_DEC // 16], i16, tag="ibuf2", bufs=3)
                        aft2 = dcp.tile([9, GATHER_CHUNK_DEC], dtt, tag="aft2", bufs=2)
                        d64 = dcp.tile([BLK, GATHER_CHUNK_DEC], dtt, tag="d64", bufs=2)
                        ohT = dcp.tile([BLK, GATHER_CHUNK_DEC], dtt, tag="ohT", bufs=2)
                        nc.sync.dma_start(iu[:, :n // 16],
                                          idx_layer[:, o // 16:(o + n) // 16])
                        nc.sync.dma_start(aft2[:, :n], afT_d[:, o:o + n])
                        nc.sync.dma_start(d64[:, :n], dstc64_d[:, o:o + n])
                        iotac_b = ct["iotacol"][:].broadcast_to([BLK, n])
                        nc.vector.tensor_tensor(ohT[:, :n], d64[:, :n], iotac_b,
                                                op=Alu.is_equal)
                        src_ap = tabs[L][0:SPLIT, :] if o < S_lo else tabs[L][SPLIT:RT, :]
                        if "nogather" not in abl and "nogu" not in abl:
                            # non-transposed gather [slot, H] (transposed
                            # gathers corrupt each other when overlapped)
                            nc.gpsimd.dma_gather(
                                gu[:, :n // 128, :], src_ap, iu[:, :n // 16],
                                num_idxs=n, num_idxs_reg=n, elem_size=H,
                                queue_num=0, single_packet=False)
                        else:
                            # same bytes, contiguous: isolates gather cost
                            nc.sync.dma_start(
                                gu[:, :n // 128, :],
                                tabs[L][0:n, :].rearrange("(p r) h -> p r h", p=128))
                        hmid = dcp.tile([128, GATHER_CHUNK_DEC], dtt, tag="hmid", bufs=2)
                        for (od, nd) in _chunks(n, 0, DEC_CHUNK):
                            psd = psd_pool.tile([128, DEC_CHUNK], f32, tag="ps_dec")
                            for tt in range(nd // 128):
                                t0 = (o + od) // 128 + tt
                                b = int(tile_block[t0])
                                sl = slice(od + tt * 128, od + (tt + 1) * 128)
                                sl2 = slice(tt * 128, (tt + 1) * 128)
                                nc.tensor.matmul(psd[:, sl2],
                                                 V_blk[0:BLK, b, :],
                                                 ohT[0:BLK, sl],
                                                 start=True, stop=False)
                                nc.tensor.matmul(psd[:, sl2], ct["Wd1caug"][:],
                                                 aft2[:, sl],
                                                 start=False, stop=False)
                                # += U[src]^T via identity (transposes gu tile)
                                nc.tensor.matmul(psd[:, sl2],
                                                 gu[:, (od + tt * 128) // 128, :],
                                                 ct["ident16"][:],
                                                 start=False, stop=True)
                            nc.scalar.activation(hmid[:, od:od + nd], psd[:, :nd],
                                                 Act.Relu)
                        orow = dcp.tile([1, GATHER_CHUNK_DEC], f32, tag="orow", bufs=2)
                        for (od, nd) in _chunks(n, 0, DEC_CHUNK):
                            pso = pso_pool.tile([1, DEC_CHUNK], f32, tag="ps_out")
                            nc.tensor.matmul(pso[:, :nd], ct["wd2col"][:],
                                             hmid[:, od:od + nd],
                                             start=True, stop=True)
                            nc.scalar.copy(orow[:, od:od + nd], pso[:, :nd])
                        nc.sync.dma_start(out_d[0:1, o:o + n], orow[:, :n])

    nc.compile()

    # Post-compile SWDGE queue assignment. Constraints learned the hard way:
    # (1) each DMASW completion sem (assigned positionally mod 8 over the
    #     final scheduled order) may only ever be incremented from ONE queue;
    # (2) concurrent SBUF-source gathers cross-contaminate their outputs, so
    #     all of them must share one queue (per-engine ring order serializes
    #     them); HBM-source gathers can spread across queues.
    # Assign queue PER SEM: 0 if any user is SBUF-source, else lane % NQ.
    from concourse.tile_scheduler import DMAInst
    from concourse import bass_isa as _bass_isa
    sem_users = {}
    for bb in nc.m.functions[0].blocks:
        for inst in bb.instructions:
            if (isinstance(inst, DMAInst)
                    and not isinstance(inst, _bass_isa.UserSyncedRemoteDMADescs)
                    and inst.engine == mybir.EngineType.Pool):
                upd = inst.sync_info.on_update[0]
                sem_users.setdefault(upd.ant_name, []).append(inst)
    for name, insts in sem_users.items():
        lane = int(name[len("DMASW"):].split("_")[0])
        if any(getattr(i, "sbuf_tokens_per_rank", 0) > 0 for i in insts):
            q = 0
        else:
            q = lane % NQ
        for i in insts:
            i.queue_num = q
    return nc


_CACHE = {}
DEBUG = False
TRACE = False
LAST = {}


def kernel(**inputs) -> np.ndarray:
    from concourse.bass_utils import run_bass_kernel_spmd

    meta, consts, per_core, bd2 = _prep(inputs)

    key = (meta["Epad"], tuple(meta["B"].tolist()), DEBUG)
    if key not in _CACHE:
        _CACHE[key] = _build(meta, dbg=DEBUG)
    nc = _CACHE[key]

    in_maps = []
    for c in range(C):
        pc = per_core[c]
        m = dict(consts)
        for k in ["h_oldT", "idx_layer", "dstc", "dstc64", "afT"]:
            m[k] = pc[k]
        in_maps.append(m)

    res = run_bass_kernel_spmd(nc, in_maps, core_ids=list(range(C)), trace=TRACE)
    LAST["exec_time_ns"] = res.exec_time_ns
    if DEBUG:
        LAST["res"] = res.results
        LAST["per_core"] = per_core
        LAST["meta"] = meta

    out = np.empty((E, 1), np.float32)
    for c in range(C):
        vals = res.results[c]["out"][0]      # [Epad], layer slot order
        inv = per_core[c]["inv"]
        good = inv >= 0
        out[inv[good], 0] = vals[good] + bd2[0]
    return out



# revision 38
# speedup vs baseline: 1.3492x; 1.3492x over previous
"""Trainium2 Bass kernel for NodeCentricNewReasoner GNN (3-layer message passing).

Strategy: shard edges by dst-node range across 8 cores (6250 nodes/core).
Each layer: AllGather node features (bf16) to an HBM table -> dma_gather rows
per edge -> segment-sum via one-hot matmuls accumulated in PSUM (f32) ->
node update (GraphConv + LayerNorm + ReLU + residual, f32) on own nodes.
Decoder: SBUF-source transposed dma_gather of final x for src/dst, fully
matmul-based MLP (no per-edge vector ops).
"""

import numpy as np

N = 50000
E = 800000
H = 128
L = 3
C = 8                      # cores
NPC = N // C               # 6250 own nodes per core
BLK = 64                   # nodes per scatter block (one-hot matmul width)
NPADC = 6272               # own nodes padded (98 * 64)
NBLK = NPADC // BLK        # 98
RT = C * NPADC             # 50176 table rows
SPLIT = 32768              # int16 index limit split
EPS = 1e-5

GATHER_CHUNK = 4096        # slots per dma_gather call (layer phase)
GB_BUFS = 5                # gather buffer rotation depth (layer phase)
GATHER_CHUNK_DEC = 2048    # slots per dma_gather call (decoder phase)
DEC_CHUNK = 512            # decoder psum chunk (slots)
BF16 = True                # bf16 tables/gathers/matmul inputs (f32 accumulate)
NQ = 4                     # swdge queues round-robin


def _wrap_idx(vals):
    """int32 [n] -> wrapped int16 [128, n//16] (16-part wrap, replicated x8)."""
    n = len(vals)
    assert n % 16 == 0
    w = vals.astype(np.int16).reshape(n // 16, 16).T  # [16, n//16]
    return np.ascontiguousarray(np.tile(w, (8, 1)))


def _dt_tab():
    if BF16:
        return np.dtype(np.float16)
    return np.dtype(np.float32)


def _prep(inputs):
    """Host-side preprocessing: sharding, slot layout, weights packing."""
    f32 = np.float32
    src = np.asarray(inputs["edge_index_new"][0]).astype(np.int64)
    dst = np.asarray(inputs["edge_index_new"][1]).astype(np.int64)
    af = np.asarray(inputs["aligned_features"]).astype(f32)
    h_old = np.asarray(inputs["h_nodes_old"]).astype(f32)

    Wf = np.asarray(inputs["Wf"]).astype(f32)
    bf = np.asarray(inputs["bf"]).astype(f32)
    We1 = np.asarray(inputs["We1"]).astype(f32)
    be1 = np.asarray(inputs["be1"]).astype(f32)
    We2 = np.asarray(inputs["We2"]).astype(f32)
    be2 = np.asarray(inputs["be2"]).astype(f32)
    Wrel = np.asarray(inputs["Wrel"]).astype(f32)
    brel = np.asarray(inputs["brel"]).astype(f32)
    Wroot = np.asarray(inputs["Wroot"]).astype(f32)
    gamma = np.asarray(inputs["gamma"]).astype(f32)
    beta = np.asarray(inputs["beta"]).astype(f32)
    Wd1 = np.asarray(inputs["Wd1"]).astype(f32)
    bd1 = np.asarray(inputs["bd1"]).astype(f32)
    Wd2 = np.asarray(inputs["Wd2"]).astype(f32)
    bd2 = np.asarray(inputs["bd2"]).astype(f32)

    core = dst // NPC
    loc = (dst - core * NPC).astype(np.int64)          # local dst in [0, 6250)
    blk = loc // BLK
    col = loc % BLK
    sr = ((src // NPC) * NPADC + (src % NPC)).astype(np.int64)  # src table row
    half = (sr >= SPLIT).astype(np.int64)
    assert NPC < SPLIT  # local dst indices fit the int16 gather index format

    # ---- layer slot layout: group by (src_half, dst_block), budgets = max over cores
    NG = 2 * NBLK
    key = half * NBLK + blk                             # [E]
    cnt = np.zeros((C, NG), np.int64)
    for c in range(C):
        cnt[c] = np.bincount(key[core == c], minlength=NG)
    B = np.maximum(128, ((cnt.max(axis=0) + 127) // 128) * 128)  # [NG] slots per group
    offs = np.zeros(NG + 1, np.int64)
    offs[1:] = np.cumsum(B)
    S_lo = int(B[:NBLK].sum())
    S_hi = int(B[NBLK:].sum())
    Epad = S_lo + S_hi
    T = Epad // 128

    # per-tile static structure
    tile_group = np.repeat(np.arange(NG), B // 128)     # [T]
    tile_block = tile_group % NBLK
    tg = tile_group
    t_start = np.zeros(T, bool)
    t_stop = np.zeros(T, bool)
    t_start[0] = True
    t_start[1:] = tg[1:] != tg[:-1]
    t_stop[-1] = True
    t_stop[:-1] = tg[1:] != tg[:-1]
    grp_is_hi = np.arange(NG) >= NBLK

    # ---- per-core slot data
    per_core = []
    for c in range(C):
        m = core == c
        k_c = key[m]
        # secondary sort by src row: gathers read ascending addresses
        # within each group (DRAM locality)
        order = np.lexsort((sr[m], k_c))
        ksort = k_c[order]
        grp_first = np.searchsorted(ksort, np.arange(NG), side="left")
        pos_sorted = np.arange(len(ksort)) - grp_first[ksort]
        slotpos = offs[ksort] + pos_sorted
        eids = np.nonzero(m)[0][order]

        slot_sr = np.zeros(Epad, np.int64)
        slot_sr[S_lo:] = SPLIT                          # hi-region padding default
        slot_dc = np.full(Epad, 127.0, f32)             # 127 => one-hot always 0
        slot_loc = np.zeros(Epad, np.int64)             # local dst row (V gather)
        slot_af = np.zeros((Epad, 8), f32)
        slot_sr[slotpos] = sr[eids]
        slot_dc[slotpos] = col[eids].astype(f32)
        slot_loc[slotpos] = loc[eids]
        slot_af[slotpos] = af[eids]

        inv = np.full(Epad, -1, np.int64)
        inv[slotpos] = eids

        idx_w = np.concatenate([slot_sr[:S_lo], slot_sr[S_lo:] - SPLIT])
        ho = h_old[c * NPC:(c + 1) * NPC]
        hoT = np.zeros((H, NPADC), f32)
        hoT[:, :NPC] = ho.T
        per_core.append(dict(
            idx_layer=_wrap_idx(idx_w),
            dstc=np.ascontiguousarray(slot_dc.reshape(T, 128).T),         # [128, T]
            dstc64=np.ascontiguousarray(np.broadcast_to(
                slot_dc[None, :], (BLK, Epad)).astype(_dt_tab())),         # [64, Epad]
            afT=np.ascontiguousarray(np.vstack(
                [slot_af.T, np.ones((1, Epad), f32)]).astype(_dt_tab())),  # [9, Epad]
            inv=inv,
            h_oldT=hoT,
        ))

    # ---- packed weights (shared across cores)
    c0 = bf + Wf[:H].sum(axis=0)
    consts = dict(
        Wf2=np.ascontiguousarray(Wf[H:]),                 # rhs [128,128]
        c0_row=np.tile(c0[None, :], (128, 1)),
        We1aug=np.vstack([We1, be1[None, :]]),            # [9,128]
        we2row=np.tile(We2[:, 0][None, :], (128, 1)),
        Wrel0=Wrel[0], Wrel1=Wrel[1], Wrel2=Wrel[2],
        Wroot0=Wroot[0], Wroot1=Wroot[1], Wroot2=Wroot[2],
        brel0=np.tile(brel[0][None, :], (128, 1)),
        brel1=np.tile(brel[1][None, :], (128, 1)),
        brel2=np.tile(brel[2][None, :], (128, 1)),
        gam0=np.tile(gamma[0][None, :], (128, 1)),
        gam1=np.tile(gamma[1][None, :], (128, 1)),
        gam2=np.tile(gamma[2][None, :], (128, 1)),
        bet0=np.tile(beta[0][None, :], (128, 1)),
        bet1=np.tile(beta[1][None, :], (128, 1)),
        bet2=np.tile(beta[2][None, :], (128, 1)),
        Wd1a=np.ascontiguousarray(Wd1[:H]),
        Wd1b=np.ascontiguousarray(Wd1[H:2 * H]),
        Wd1caug=np.vstack([Wd1[2 * H:], bd1[None, :]]),   # [9,128]
        wd2col=np.ascontiguousarray(Wd2[:, 0:1]),         # [128,1] Wd2 along K
        iota64=np.tile(np.arange(BLK, dtype=f32)[None, :], (128, 1)),
        iotacol=np.arange(BLK, dtype=f32).reshape(BLK, 1),
        ident16=np.eye(128, dtype=f32),
        epscol=np.full((128, 1), EPS, f32),
        be2col=np.full((128, 1), be2[0], f32),
        ident=np.eye(128, dtype=f32),
    )
    consts = {k: np.ascontiguousarray(v.astype(f32)) for k, v in consts.items()}
    for k in ("We1aug", "we2row", "Wd1a", "Wd1b", "Wd1caug", "wd2col", "ident16"):
        consts[k] = np.ascontiguousarray(consts[k].astype(_dt_tab()))

    meta = dict(
        B=B, S_lo=S_lo, S_hi=S_hi, Epad=Epad, T=T,
        tile_block=tile_block, tile_group=tile_group,
        t_start=t_start, t_stop=t_stop, grp_is_hi=grp_is_hi,
    )
    return meta, consts, per_core, bd2


def _chunks(total, start_slot, sz=None):
    sz = sz or GATHER_CHUNK
    out = []
    o = 0
    while o < total:
        n = min(sz, total - o)
        out.append((start_slot + o, n))
        o += n
    return out


def _build(meta, dbg=False, sim1=False, abl=()):
    """abl: ablation flags for timing experiments (default none):
    'rep2'/'rep3' repeat whole computation; 'nogather' skip dma_gathers;
    'nocoll' replace AllGather with local copy; 'nodec' skip decoder."""
    import concourse.bacc as bacc
    import concourse.tile as tile
    import concourse.mybir as mybir
    from concourse import library_config

    f32 = mybir.dt.float32
    i16 = mybir.dt.int16
    dtt = mybir.dt.float16 if BF16 else mybir.dt.float32
    dttn = 2 if BF16 else 4   # bytes
    Alu = mybir.AluOpType
    Act = mybir.ActivationFunctionType

    S_lo, S_hi, Epad, T = meta["S_lo"], meta["S_hi"], meta["Epad"], meta["T"]
    tile_block = meta["tile_block"]
    tile_group = meta["tile_group"]
    t_start = meta["t_start"]
    t_stop = meta["t_stop"]

    # evac plan: group g -> mode at its last tile. 0: copy into aggT, 1: add
    B = meta["B"]
    first_half_of_block = {}
    evac_mode = {}
    for g in range(2 * NBLK):
        b = g % NBLK
        if B[g] == 0:
            continue
        if b not in first_half_of_block:
            first_half_of_block[b] = g
            evac_mode[g] = 0
        else:
            evac_mode[g] = 1

    nc = bacc.Bacc("TRN2", target_bir_lowering=False, debug=False,
                   enable_asserts=False, num_devices=1 if sim1 else C,
                   num_swdge_queues=NQ)

    # SWDGE completion sems are assigned round-robin (mod 8) over ALL SWDGE
    # DMA instructions in program order (tile_sem_assignment.next_sw_dma_idx),
    # and each sem is locked to one queue. queue = global_index % NQ (NQ
    # divides 8) keeps every sem slot on a single queue.
    _gq = [0]

    def next_queue():
        q = _gq[0] % NQ
        _gq[0] += 1
        return q

    def din(name, shape, dt=f32):
        return nc.dram_tensor(name, shape, dt, kind="ExternalInput").ap()

    h_oldT = din("h_oldT", [H, NPADC])
    idx_layer = din("idx_layer", [128, Epad // 16], i16)
    dstc_d = din("dstc", [128, T])
    dstc64_d = din("dstc64", [BLK, Epad], dtt)
    afT_d = din("afT", [9, Epad], dtt)
    cn = {k: din(k, [128, 128]) for k in [
        "Wf2", "c0_row", "Wrel0", "Wrel1", "Wrel2",
        "Wroot0", "Wroot1", "Wroot2", "brel0", "brel1", "brel2",
        "gam0", "gam1", "gam2", "bet0", "bet1", "bet2", "ident"]}
    for k in ("Wd1a", "Wd1b", "we2row", "ident16"):
        cn[k] = din(k, [128, 128], dtt)
    cn["wd2col"] = din("wd2col", [128, 1], dtt)
    cn["We1aug"] = din("We1aug", [9, 128], dtt)
    cn["Wd1caug"] = din("Wd1caug", [9, 128], dtt)
    cn["iota64"] = din("iota64", [128, BLK])
    cn["iotacol"] = din("iotacol", [BLK, 1])
    cn["epscol"] = din("epscol", [128, 1])
    cn["be2col"] = din("be2col", [128, 1])
    out_d = nc.dram_tensor("out", [1, Epad], f32, kind="ExternalOutput").ap()
    dbg_d = {}
    if "dbguv" in abl:
        dbg_d["U"] = nc.dram_tensor("dbg_U", [RT, H], dtt, kind="ExternalOutput").ap()
        dbg_d["V"] = nc.dram_tensor("dbg_V", [NPADC, H], dtt, kind="ExternalOutput").ap()
    if dbg:
        for l in range(L + 1):
            dbg_d[f"x{l}"] = nc.dram_tensor(
                f"dbg_x{l}", [NPADC, H], f32, kind="ExternalOutput").ap()
        dbg_d["ew"] = nc.dram_tensor("dbg_ew", [128, T], f32, kind="ExternalOutput").ap()
        dbg_d["agg0"] = nc.dram_tensor("dbg_agg0", [H, NPADC], f32, kind="ExternalOutput").ap()

    NT = NPADC // 128  # node tiles

    with tile.TileContext(nc) as tc:
        with tc.tile_pool(name="const", bufs=1) as cpool, \
             tc.tile_pool(name="dram", bufs=1, space="DRAM") as dpool:

            nc.gpsimd.load_library(library_config.mlp)

            ct = {}
            for k, d in cn.items():
                dt_ = dtt if k in ("We1aug", "we2row", "Wd1a", "Wd1b",
                                   "Wd1caug", "wd2col", "ident16") else f32
                t_ = cpool.tile(list(d.shape), dt_, tag=f"c_{k}", name=f"c_{k}")
                nc.sync.dma_start(t_[:], d[:])
                ct[k] = t_

            agin = [dpool.tile([NPADC, H], dtt, tag=f"agin{l}", name=f"agin{l}")
                    for l in range(L + 1)]
            tabs = [dpool.tile([RT, H], dtt, tag=f"tab{l}", name=f"tab{l}")
                    for l in range(L + 1)]
            # V = x3 @ Wd1b for own nodes (written at layer-3 export, read by
            # the decoder after the layer pools are freed)
            V_dram = dpool.tile([NPADC, H], dtt, tag="V_dram", name="V_dram")

            REP = 3 if "rep3" in abl else (2 if "rep2" in abl else 1)
            for _rep in range(REP):
                # ======== layer phase pools (freed before decoder) ========
                with tc.tile_pool(name="lpersist", bufs=1) as ppool, \
                     tc.tile_pool(name="work", bufs=2) as wpool, \
                     tc.tile_pool(name="gath", bufs=1) as gpool, \
                     tc.tile_pool(name="psum", bufs=4, space="PSUM") as pspool, \
                     tc.tile_pool(name="psum2", bufs=4, space="PSUM") as ps2pool:

                    dstc = ppool.tile([128, T], f32, tag="dstc")
                    nc.sync.dma_start(dstc[:], dstc_d[:])
                    ew = ppool.tile([128, T], f32, tag="ew")
                    x_own = ppool.tile([128, NT, H], f32, tag="x_own")
                    xT_own = ppool.tile([H, NPADC], f32, tag="xT_own")
                    aggT = ppool.tile([H, NPADC], f32, tag="aggT")

                    def export_tile(l, nt):
                        # l < L: export x_l rows (gather table for layer l).
                        # l == L: export U = x3 @ Wd1a instead (decoder table),
                        # and V = x3 @ Wd1b for own nodes via DRAM.
                        pst = ps2pool.tile([128, 128], f32, tag="ps_a")
                        nc.tensor.transpose(pst[:], x_own[:, nt, :], ct["ident"][:])
                        if l < L:
                            nc.vector.tensor_copy(
                                xT_own[:, nt * 128:(nt + 1) * 128], pst[:])
                            xrow = wpool.tile([128, 128], dtt, tag="w_xrow")
                            nc.scalar.copy(xrow[:], x_own[:, nt, :])
                            nc.sync.dma_start(
                                agin[l][nt * 128:(nt + 1) * 128, :], xrow[:])
                        else:
                            xTb = wpool.tile([128, 128], dtt, tag="w_xTb")
                            nc.scalar.copy(xTb[:], pst[:])
                            psU = ps2pool.tile([128, 128], f32, tag="ps_a")
                            nc.tensor.matmul(psU[:], xTb[:], ct["Wd1a"][:],
                                             start=True, stop=True)
                            urow = wpool.tile([128, 128], dtt, tag="w_xrow")
                            nc.scalar.copy(urow[:], psU[:])
                            nc.sync.dma_start(
                                agin[l][nt * 128:(nt + 1) * 128, :], urow[:])
                            psV = ps2pool.tile([128, 128], f32, tag="ps_a")
                            nc.tensor.matmul(psV[:], xTb[:], ct["Wd1b"][:],
                                             start=True, stop=True)
                            vrow = wpool.tile([128, 128], dtt, tag="w_vrow")
                            nc.scalar.copy(vrow[:], psV[:])
                            nc.sync.dma_start(
                                V_dram[nt * 128:(nt + 1) * 128, :], vrow[:])

                    def export_collective(l):
                        if sim1 or "nocoll" in abl:
                            nc.sync.dma_start(tabs[l][0:NPADC, :], agin[l][:])
                        else:
                            nc.gpsimd.collective_compute(
                                "AllGather", mybir.AluOpType.bypass,
                                ins=[agin[l].opt()], outs=[tabs[l].opt()],
                                replica_groups=[list(range(C))],
                            )

                    def node_transpose_and_export(l):
                        for nt in range(NT):
                            export_tile(l, nt)
                        export_collective(l)

                    # ===== Phase 1: x0 = relu(h_old @ Wf2 + c0)
                    for nt in range(NT):
                        hoT_t = wpool.tile([128, 128], f32, tag="w_hoT")
                        nc.sync.dma_start(hoT_t[:], h_oldT[:, nt * 128:(nt + 1) * 128])
                        ps = ps2pool.tile([128, 128], f32, tag="ps_a")
                        nc.tensor.matmul(ps[:], hoT_t[:], ct["Wf2"][:], start=True, stop=True)
                        tmp = wpool.tile([128, 128], f32, tag="w_init")
                        nc.vector.scalar_tensor_tensor(
                            tmp[:], ps[:], 1.0, ct["c0_row"][:], op0=Alu.mult, op1=Alu.add)
                        nc.vector.tensor_scalar_max(x_own[:, nt, :], tmp[:], 0.0)
                    node_transpose_and_export(0)
                    if dbg:
                        for nt in range(NT):
                            nc.sync.dma_start(dbg_d["x0"][nt * 128:(nt + 1) * 128, :],
                                              x_own[:, nt, :])

                    # ===== Phase 2: edge weights ew
                    AFC = 4096
                    for o in range(0, Epad, AFC):
                        n = min(AFC, Epad - o)
                        aft = wpool.tile([9, AFC], dtt, tag="w_aft")
                        nc.sync.dma_start(aft[:, :n], afT_d[:, o:o + n])
                        for tt in range(n // 128):
                            t0 = o // 128 + tt
                            pse = ps2pool.tile([128, 128], f32, tag="ps_a")
                            nc.tensor.matmul(pse[:], aft[:, tt * 128:(tt + 1) * 128],
                                             ct["We1aug"][:], start=True, stop=True)
                            h1 = wpool.tile([128, 128], dtt, tag="w_h1")
                            nc.scalar.activation(h1[:], pse[:], Act.Relu)
                            scr = wpool.tile([128, 128], f32, tag="w_scr")
                            nc.vector.scalar_tensor_tensor(
                                scr[:], h1[:], 1.0, ct["we2row"][:],
                                op0=Alu.mult, op1=Alu.mult,
                                accum_out=ew[:, t0:t0 + 1])
                    nc.scalar.activation(ew[:], ew[:], Act.Sigmoid, bias=ct["be2col"][:, 0:1])
                    if dbg:
                        nc.sync.dma_start(dbg_d["ew"][:], ew[:])

                    # ===== Phase 3: layers
                    def node_update(l, nt):
                        Wrel_t, Wroot_t = ct[f"Wrel{l}"], ct[f"Wroot{l}"]
                        brel_t, gam_t, bet_t = ct[f"brel{l}"], ct[f"gam{l}"], ct[f"bet{l}"]
                        sl = slice(nt * 128, (nt + 1) * 128)
                        psn = ps2pool.tile([128, 128], f32, tag="ps_a")
                        nc.tensor.matmul(psn[:], aggT[:, sl], Wrel_t[:],
                                         start=True, stop=False)
                        nc.tensor.matmul(psn[:], xT_own[:, sl], Wroot_t[:],
                                         start=False, stop=True)
                        hsb = wpool.tile([128, 128], f32, tag="w_h")
                        nc.vector.scalar_tensor_tensor(
                            hsb[:], psn[:], 1.0, brel_t[:], op0=Alu.mult, op1=Alu.add)
                        mu = wpool.tile([128, 1], f32, tag="w_mu")
                        nc.vector.tensor_reduce(mu[:], hsb[:],
                                                axis=mybir.AxisListType.X, op=Alu.add)
                        nc.scalar.mul(mu[:], mu[:], 1.0 / H)
                        cent = wpool.tile([128, 128], f32, tag="w_cent")
                        nc.vector.tensor_scalar(cent[:], hsb[:], mu[:], None,
                                                op0=Alu.subtract)
                        sq = wpool.tile([128, 128], f32, tag="w_sq")
                        vs = wpool.tile([128, 1], f32, tag="w_vs")
                        nc.scalar.activation(sq[:], cent[:], Act.Square, accum_out=vs[:])
                        std = wpool.tile([128, 1], f32, tag="w_std")
                        nc.scalar.activation(std[:], vs[:], Act.Sqrt,
                                             bias=ct["epscol"][:, 0:1], scale=1.0 / H)
                        rstd = wpool.tile([128, 1], f32, tag="w_rstd")
                        nc.vector.reciprocal(rstd[:], std[:])
                        t1 = wpool.tile([128, 128], f32, tag="w_t1")
                        nc.vector.scalar_tensor_tensor(
                            t1[:], cent[:], rstd[:], gam_t[:], op0=Alu.mult, op1=Alu.mult)
                        t2 = wpool.tile([128, 128], f32, tag="w_t2")
                        nc.vector.tensor_tensor(t2[:], t1[:], bet_t[:], op=Alu.add)
                        nc.vector.scalar_tensor_tensor(
                            x_own[:, nt, :], t2[:], 0.0, x_own[:, nt, :],
                            op0=Alu.max, op1=Alu.add)

                    # group-aligned gather chunks: PSUM accumulation chains
                    # never cross a chunk boundary
                    offs_b = np.zeros(2 * NBLK + 1, np.int64)
                    offs_b[1:] = np.cumsum(B)

                    def group_plan(g_lo, g_hi):
                        chunks = []
                        cur_o, cur_n = None, 0
                        for g in range(g_lo, g_hi):
                            bg = int(B[g])
                            assert bg <= GATHER_CHUNK
                            if cur_o is None:
                                cur_o, cur_n = int(offs_b[g]), 0
                            elif cur_n + bg > GATHER_CHUNK:
                                chunks.append((cur_o, cur_n))
                                cur_o, cur_n = int(offs_b[g]), 0
                            cur_n += bg
                        if cur_n:
                            chunks.append((cur_o, cur_n))
                        return chunks

                    gplan = group_plan(0, NBLK) + group_plan(NBLK, 2 * NBLK)

                    for l in range(L):
                        qi = 0
                        ps_blk = None
                        for (o, n) in gplan:
                            TT = n // 128
                            t0w = o // 128
                            gb = gpool.tile([128, GATHER_CHUNK // 128, H], dtt,
                                            tag="gbuf", bufs=GB_BUFS)
                            iw = gpool.tile([128, GATHER_CHUNK // 16], i16, tag="ibuf", bufs=6)
                            nc.sync.dma_start(iw[:, :n // 16],
                                              idx_layer[:, o // 16:(o + n) // 16])
                            src_ap = tabs[l][0:SPLIT, :] if o < S_lo else tabs[l][SPLIT:RT, :]
                            if "nogather" not in abl:
                                nc.gpsimd.dma_gather(
                                    gb[:, :TT, :], src_ap, iw[:, :n // 16],
                                    num_idxs=n, num_idxs_reg=n, elem_size=H,
                                    queue_num=next_queue(), single_packet=False)
                            else:
                                # same bytes, contiguous: isolates desc-gen cost
                                nc.sync.dma_start(
                                    gb[:, :TT, :],
                                    tabs[l][0:n, :].rearrange(
                                        "(p r) h -> p r h", p=128))
                            qi += 1
                            # one-hot * ew for the whole chunk: 2 wide DVE ops
                            eqw = gpool.tile([128, GATHER_CHUNK // 128, BLK], dtt,
                                             tag="eqw", bufs=3)
                            ohew = gpool.tile([128, GATHER_CHUNK // 128, BLK], dtt,
                                              tag="ohew", bufs=3)
                            dstc_b = dstc[:, t0w:t0w + TT].rearrange(
                                "p (t u) -> p t u", u=1).broadcast_to([128, TT, BLK])
                            ew_b = ew[:, t0w:t0w + TT].rearrange(
                                "p (t u) -> p t u", u=1).broadcast_to([128, TT, BLK])
                            iota_b = ct["iota64"][:].rearrange(
                                "p (u b) -> p u b", u=1).broadcast_to([128, TT, BLK])
                            nc.vector.tensor_tensor(eqw[:, :TT, :], dstc_b, iota_b,
                                                    op=Alu.is_equal)
                            nc.vector.tensor_tensor(ohew[:, :TT, :], eqw[:, :TT, :],
                                                    ew_b, op=Alu.mult)
                            for tt in range(TT):
                                t0 = t0w + tt
                                g = int(tile_group[t0])
                                b = int(tile_block[t0])
                                if t_start[t0]:
                                    ps_blk = pspool.tile([128, BLK], f32, tag="ps_sc")
                                nc.tensor.matmul(ps_blk[:], gb[:, tt, :], ohew[:, tt, :],
                                                 start=bool(t_start[t0]),
                                                 stop=bool(t_stop[t0]))
                                if t_stop[t0]:
                                    sl = aggT[:, b * BLK:(b + 1) * BLK]
                                    if evac_mode[g] == 0:
                                        nc.vector.tensor_copy(sl, ps_blk[:])
                                    else:
                                        nc.vector.tensor_tensor(sl, ps_blk[:], sl, op=Alu.add)
                                    # interleave node updates into the hi-pass:
                                    # pair (2nt, 2nt+1) is final once the hi
                                    # group of block 2nt+1 has been evacuated
                                    if g >= NBLK and b % 2 == 1:
                                        node_update(l, b // 2)
                                        export_tile(l + 1, b // 2)
                        if dbg and l == 0:
                            nc.sync.dma_start(dbg_d["agg0"][:, :], aggT[:, :])
                        export_collective(l + 1)
                        if dbg:
                            for nt in range(NT):
                                nc.sync.dma_start(
                                    dbg_d[f"x{l + 1}"][nt * 128:(nt + 1) * 128, :],
                                    x_own[:, nt, :])
                # lpersist + gath pools freed here

                # ========== Phase 4: decoder (reuses layer slot layout) =========
                # out[e] = wd2 . relu(U[src_e] + V[dst_e] + af_e @ Wd1c + bd1)
                # U[src]: HBM transposed gather from tabs[L] (concurrent-safe
                # across queues, unlike SBUF-source gathers).
                # V[dst]: slots are dst-block-grouped, so V comes from one-hot
                # matmuls against SBUF-resident V_blk; the one-hot [64, n] is
                # built on device from the streamed dstc64 row values.
                with tc.tile_pool(name="dec", bufs=1) as dcp, \
                     tc.tile_pool(name="psdec", bufs=3, space="PSUM") as psd_pool, \
                     tc.tile_pool(name="psout", bufs=4, space="PSUM") as pso_pool:
                    # V per 64-row dst block, base partition 0
                    V_blk = dcp.tile([BLK, NBLK, H], dtt, tag="V_blk")
                    nc.sync.dma_start(
                        V_blk[:, :, :],
                        V_dram[:].rearrange("(b r) h -> r b h", r=BLK))

                    if "dbguv" in abl:
                        nc.sync.dma_start(dbg_d["U"][:, :], tabs[L][:, :])
                        nc.sync.dma_start(dbg_d["V"][:, :], V_dram[:, :])

                    plan = (_chunks(S_lo, 0, GATHER_CHUNK_DEC)
                            + _chunks(S_hi, S_lo, GATHER_CHUNK_DEC))
                    if "nodec" in abl:
                        plan = []
                    for (o, n) in plan:
                        gu = dcp.tile([128, GATHER_CHUNK_DEC // 128, H], dtt,
                                      tag="gu", bufs=3)
                        iu = dcp.tile([128, GATHER_CHUNK_DEC // 16], i16, tag="ibuf2", bufs=3)
                        aft2 = dcp.tile([9, GATHER_CHUNK_DEC], dtt, tag="aft2", bufs=2)
                        d64 = dcp.tile([BLK, GATHER_CHUNK_DEC], dtt, tag="d64", bufs=2)
                        ohT = dcp.tile([BLK, GATHER_CHUNK_DEC], dtt, tag="ohT", bufs=2)
                        nc.sync.dma_start(iu[:, :n // 16],
                                          idx_layer[:, o // 16:(o + n) // 16])
                        nc.sync.dma_start(aft2[:, :n], afT_d[:, o:o + n])
                        nc.sync.dma_start(d64[:, :n], dstc64_d[:, o:o + n])
                        iotac_b = ct["iotacol"][:].broadcast_to([BLK, n])
                        nc.vector.tensor_tensor(ohT[:, :n], d64[:, :n], iotac_b,
                                                op=Alu.is_equal)
                        src_ap = tabs[L][0:SPLIT, :] if o < S_lo else tabs[L][SPLIT:RT, :]
                        if "nogather" not in abl and "nogu" not in abl:
                            # non-transposed gather [slot, H] (transposed
                            # gathers corrupt each other when overlapped)
                            nc.gpsimd.dma_gather(
                                gu[:, :n // 128, :], src_ap, iu[:, :n // 16],
                                num_idxs=n, num_idxs_reg=n, elem_size=H,
                                queue_num=0, single_packet=False)
                        else:
                            # same bytes, contiguous: isolates gather cost
                            nc.sync.dma_start(
                                gu[:, :n // 128, :],
                                tabs[L][0:n, :].rearrange("(p r) h -> p r h", p=128))
                        hmid = dcp.tile([128, GATHER_CHUNK_DEC], dtt, tag="hmid", bufs=2)
                        for (od, nd) in _chunks(n, 0, DEC_CHUNK):
                            psd = psd_pool.tile([128, DEC_CHUNK], f32, tag="ps_dec")
                            for tt in range(nd // 128):
                                t0 = (o + od) // 128 + tt
                                b = int(tile_block[t0])
                                sl = slice(od + tt * 128, od + (tt + 1) * 128)
                                sl2 = slice(tt * 128, (tt + 1) * 128)
                                nc.tensor.matmul(psd[:, sl2],
                                                 V_blk[0:BLK, b, :],
                                                 ohT[0:BLK, sl],
                                                 start=True, stop=False)
                                nc.tensor.matmul(psd[:, sl2], ct["Wd1caug"][:],
                                                 aft2[:, sl],
                                                 start=False, stop=False)
                                # += U[src]^T via identity (transposes gu tile)
                                nc.tensor.matmul(psd[:, sl2],
                                                 gu[:, (od + tt * 128) // 128, :],
                                                 ct["ident16"][:],
                                                 start=False, stop=True)
                            nc.scalar.activation(hmid[:, od:od + nd], psd[:, :nd],
                                                 Act.Relu)
                        orow = dcp.tile([1, GATHER_CHUNK_DEC], f32, tag="orow", bufs=2)
                        for (od, nd) in _chunks(n, 0, DEC_CHUNK):
                            pso = pso_pool.tile([1, DEC_CHUNK], f32, tag="ps_out")
                            nc.tensor.matmul(pso[:, :nd], ct["wd2col"][:],
                                             hmid[:, od:od + nd],
                                             start=True, stop=True)
                            nc.scalar.copy(orow[:, od:od + nd], pso[:, :nd])
                        nc.sync.dma_start(out_d[0:1, o:o + n], orow[:, :n])

    nc.compile()

    # Post-compile SWDGE queue assignment. Constraints learned the hard way:
    # (1) each DMASW completion sem (assigned positionally mod 8 over the
    #     final scheduled order) may only ever be incremented from ONE queue;
    # (2) concurrent SBUF-source gathers cross-contaminate their outputs, so
    #     all of them must share one queue (per-engine ring order serializes
    #     them); HBM-source gathers can spread across queues.
    # Assign queue PER SEM: 0 if any user is SBUF-source, else lane % NQ.
    from concourse.tile_scheduler import DMAInst
    from concourse import bass_isa as _bass_isa
    sem_users = {}
    for bb in nc.m.functions[0].blocks:
        for inst in bb.instructions:
            if (isinstance(inst, DMAInst)
                    and not isinstance(inst, _bass_isa.UserSyncedRemoteDMADescs)
                    and inst.engine == mybir.EngineType.Pool):
                upd = inst.sync_info.on_update[0]
                sem_users.setdefault(upd.ant_name, []).append(inst)
    for name, insts in sem_users.items():
        lane = int(name[len("DMASW"):].split("_")[0])
        if any(getattr(i, "sbuf_tokens_per_rank", 0) > 0 for i in insts):
            q = 0
        else:
            q = lane % NQ
        for i in insts:
            i.queue_num = q
    return nc


_CACHE = {}
DEBUG = False
TRACE = False
LAST = {}


def kernel(**inputs) -> np.ndarray:
    from concourse.bass_utils import run_bass_kernel_spmd

    meta, consts, per_core, bd2 = _prep(inputs)

    key = (meta["Epad"], tuple(meta["B"].tolist()), DEBUG)
    if key not in _CACHE:
        _CACHE[key] = _build(meta, dbg=DEBUG)
    nc = _CACHE[key]

    in_maps = []
    for c in range(C):
        pc = per_core[c]
        m = dict(consts)
        for k in ["h_oldT", "idx_layer", "dstc", "dstc64", "afT"]:
            m[k] = pc[k]
        in_maps.append(m)

    res = run_bass_kernel_spmd(nc, in_maps, core_ids=list(range(C)), trace=TRACE)
    LAST["exec_time_ns"] = res.exec_time_ns
    if DEBUG:
        LAST["res"] = res.results
        LAST["per_core"] = per_core
        LAST["meta"] = meta

    out = np.empty((E, 1), np.float32)
    for c in range(C):
        vals = res.results[c]["out"][0]      # [Epad], layer slot order
        inv = per_core[c]["inv"]
        good = inv >= 0
        out[inv[good], 0] = vals[good] + bd2[0]
    return out



# revision 39
# speedup vs baseline: 1.3618x; 1.0093x over previous
"""Trainium2 Bass kernel for NodeCentricNewReasoner GNN (3-layer message passing).

Strategy: shard edges by dst-node range across 8 cores (6250 nodes/core).
Each layer: AllGather node features (bf16) to an HBM table -> dma_gather rows
per edge (group-aligned chunks) -> segment-sum via one-hot matmuls accumulated
in PSUM (f32); the one-hot*ew tiles are built with 2 wide DVE ops per chunk
(stride-0 broadcast APs) -> node update (GraphConv + LayerNorm + ReLU +
residual, f32) on own nodes, interleaved into the hi-half pass.
Decoder: U[src] via non-transposed HBM dma_gather of the U=x3@Wd1a table
(transposed into PSUM with an identity matmul); V[dst] via one-hot matmuls
against SBUF-resident V_blk (one-hot built on device from streamed dstc64);
everything pre-relu accumulates in one PSUM pass.

Hardware constraints baked in (found the hard way):
- DMASW completion sems are assigned positionally (mod 8) over the final
  scheduled order; queue_num must be consistent per sem -> post-compile fixup.
- Transposed dma_gathers (HBM- or SBUF-source) corrupt each other when more
  than one is in flight; only non-transposed HBM gathers overlap safely.
"""

import numpy as np

N = 50000
E = 800000
H = 128
L = 3
C = 8                      # cores
NPC = N // C               # 6250 own nodes per core
BLK = 64                   # nodes per scatter block (one-hot matmul width)
NPADC = 6272               # own nodes padded (98 * 64)
NBLK = NPADC // BLK        # 98
RT = C * NPADC             # 50176 table rows
SPLIT = 32768              # int16 index limit split
EPS = 1e-5

GATHER_CHUNK = 4096        # slots per dma_gather call (layer phase)
GB_BUFS = 5                # gather buffer rotation depth (layer phase)
GATHER_CHUNK_DEC = 2048    # slots per dma_gather call (decoder phase)
DEC_CHUNK = 512            # decoder psum chunk (slots)
BF16 = True                # bf16 tables/gathers/matmul inputs (f32 accumulate)
NQ = 4                     # swdge queues round-robin


def _wrap_idx(vals):
    """int32 [n] -> wrapped int16 [128, n//16] (16-part wrap, replicated x8)."""
    n = len(vals)
    assert n % 16 == 0
    w = vals.astype(np.int16).reshape(n // 16, 16).T  # [16, n//16]
    return np.ascontiguousarray(np.tile(w, (8, 1)))


def _dt_tab():
    if BF16:
        return np.dtype(np.float16)
    return np.dtype(np.float32)


def _prep(inputs):
    """Host-side preprocessing: sharding, slot layout, weights packing."""
    f32 = np.float32
    src = np.asarray(inputs["edge_index_new"][0]).astype(np.int64)
    dst = np.asarray(inputs["edge_index_new"][1]).astype(np.int64)
    af = np.asarray(inputs["aligned_features"]).astype(f32)
    h_old = np.asarray(inputs["h_nodes_old"]).astype(f32)

    Wf = np.asarray(inputs["Wf"]).astype(f32)
    bf = np.asarray(inputs["bf"]).astype(f32)
    We1 = np.asarray(inputs["We1"]).astype(f32)
    be1 = np.asarray(inputs["be1"]).astype(f32)
    We2 = np.asarray(inputs["We2"]).astype(f32)
    be2 = np.asarray(inputs["be2"]).astype(f32)
    Wrel = np.asarray(inputs["Wrel"]).astype(f32)
    brel = np.asarray(inputs["brel"]).astype(f32)
    Wroot = np.asarray(inputs["Wroot"]).astype(f32)
    gamma = np.asarray(inputs["gamma"]).astype(f32)
    beta = np.asarray(inputs["beta"]).astype(f32)
    Wd1 = np.asarray(inputs["Wd1"]).astype(f32)
    bd1 = np.asarray(inputs["bd1"]).astype(f32)
    Wd2 = np.asarray(inputs["Wd2"]).astype(f32)
    bd2 = np.asarray(inputs["bd2"]).astype(f32)

    core = dst // NPC
    loc = (dst - core * NPC).astype(np.int64)          # local dst in [0, 6250)
    blk = loc // BLK
    col = loc % BLK
    sr = ((src // NPC) * NPADC + (src % NPC)).astype(np.int64)  # src table row
    half = (sr >= SPLIT).astype(np.int64)
    assert NPC < SPLIT  # local dst indices fit the int16 gather index format

    # ---- layer slot layout: group by (src_half, dst_block), budgets = max over cores
    NG = 2 * NBLK
    key = half * NBLK + blk                             # [E]
    cnt = np.zeros((C, NG), np.int64)
    for c in range(C):
        cnt[c] = np.bincount(key[core == c], minlength=NG)
    B = np.maximum(128, ((cnt.max(axis=0) + 127) // 128) * 128)  # [NG] slots per group
    offs = np.zeros(NG + 1, np.int64)
    offs[1:] = np.cumsum(B)
    S_lo = int(B[:NBLK].sum())
    S_hi = int(B[NBLK:].sum())
    Epad = S_lo + S_hi
    T = Epad // 128

    # per-tile static structure
    tile_group = np.repeat(np.arange(NG), B // 128)     # [T]
    tile_block = tile_group % NBLK
    tg = tile_group
    t_start = np.zeros(T, bool)
    t_stop = np.zeros(T, bool)
    t_start[0] = True
    t_start[1:] = tg[1:] != tg[:-1]
    t_stop[-1] = True
    t_stop[:-1] = tg[1:] != tg[:-1]
    grp_is_hi = np.arange(NG) >= NBLK

    # ---- per-core slot data
    per_core = []
    for c in range(C):
        m = core == c
        k_c = key[m]
        # secondary sort by src row: gathers read ascending addresses
        # within each group (DRAM locality)
        order = np.lexsort((sr[m], k_c))
        ksort = k_c[order]
        grp_first = np.searchsorted(ksort, np.arange(NG), side="left")
        pos_sorted = np.arange(len(ksort)) - grp_first[ksort]
        slotpos = offs[ksort] + pos_sorted
        eids = np.nonzero(m)[0][order]

        slot_sr = np.zeros(Epad, np.int64)
        slot_sr[S_lo:] = SPLIT                          # hi-region padding default
        slot_dc = np.full(Epad, 127.0, f32)             # 127 => one-hot always 0
        slot_loc = np.zeros(Epad, np.int64)             # local dst row (V gather)
        slot_af = np.zeros((Epad, 8), f32)
        slot_sr[slotpos] = sr[eids]
        slot_dc[slotpos] = col[eids].astype(f32)
        slot_loc[slotpos] = loc[eids]
        slot_af[slotpos] = af[eids]

        inv = np.full(Epad, -1, np.int64)
        inv[slotpos] = eids

        idx_w = np.concatenate([slot_sr[:S_lo], slot_sr[S_lo:] - SPLIT])
        ho = h_old[c * NPC:(c + 1) * NPC]
        hoT = np.zeros((H, NPADC), f32)
        hoT[:, :NPC] = ho.T
        per_core.append(dict(
            idx_layer=_wrap_idx(idx_w),
            dstc=np.ascontiguousarray(slot_dc.reshape(T, 128).T),         # [128, T]
            dstc64=np.ascontiguousarray(np.broadcast_to(
                slot_dc[None, :], (BLK, Epad)).astype(_dt_tab())),         # [64, Epad]
            afT=np.ascontiguousarray(np.vstack(
                [slot_af.T, np.ones((1, Epad), f32)]).astype(_dt_tab())),  # [9, Epad]
            inv=inv,
            h_oldT=hoT,
        ))

    # ---- packed weights (shared across cores)
    c0 = bf + Wf[:H].sum(axis=0)
    consts = dict(
        Wf2=np.ascontiguousarray(Wf[H:]),                 # rhs [128,128]
        c0_row=np.tile(c0[None, :], (128, 1)),
        We1aug=np.vstack([We1, be1[None, :]]),            # [9,128]
        we2row=np.tile(We2[:, 0][None, :], (128, 1)),
        Wrel0=Wrel[0], Wrel1=Wrel[1], Wrel2=Wrel[2],
        Wroot0=Wroot[0], Wroot1=Wroot[1], Wroot2=Wroot[2],
        brel0=np.tile(brel[0][None, :], (128, 1)),
        brel1=np.tile(brel[1][None, :], (128, 1)),
        brel2=np.tile(brel[2][None, :], (128, 1)),
        gam0=np.tile(gamma[0][None, :], (128, 1)),
        gam1=np.tile(gamma[1][None, :], (128, 1)),
        gam2=np.tile(gamma[2][None, :], (128, 1)),
        bet0=np.tile(beta[0][None, :], (128, 1)),
        bet1=np.tile(beta[1][None, :], (128, 1)),
        bet2=np.tile(beta[2][None, :], (128, 1)),
        Wd1a=np.ascontiguousarray(Wd1[:H]),
        Wd1b=np.ascontiguousarray(Wd1[H:2 * H]),
        Wd1caug=np.vstack([Wd1[2 * H:], bd1[None, :]]),   # [9,128]
        wd2col=np.ascontiguousarray(Wd2[:, 0:1]),         # [128,1] Wd2 along K
        iota64=np.tile(np.arange(BLK, dtype=f32)[None, :], (128, 1)),
        iotacol=np.arange(BLK, dtype=f32).reshape(BLK, 1),
        ident16=np.eye(128, dtype=f32),
        epscol=np.full((128, 1), EPS, f32),
        be2col=np.full((128, 1), be2[0], f32),
        ident=np.eye(128, dtype=f32),
    )
    consts = {k: np.ascontiguousarray(v.astype(f32)) for k, v in consts.items()}
    for k in ("We1aug", "we2row", "Wd1a", "Wd1b", "Wd1caug", "wd2col", "ident16"):
        consts[k] = np.ascontiguousarray(consts[k].astype(_dt_tab()))

    meta = dict(
        B=B, S_lo=S_lo, S_hi=S_hi, Epad=Epad, T=T,
        tile_block=tile_block, tile_group=tile_group,
        t_start=t_start, t_stop=t_stop, grp_is_hi=grp_is_hi,
    )
    return meta, consts, per_core, bd2


def _chunks(total, start_slot, sz=None):
    sz = sz or GATHER_CHUNK
    out = []
    o = 0
    while o < total:
        n = min(sz, total - o)
        out.append((start_slot + o, n))
        o += n
    return out


def _build(meta, dbg=False, sim1=False, abl=()):
    """abl: ablation flags for timing experiments (default none):
    'rep2'/'rep3' repeat whole computation; 'nogather' skip dma_gathers;
    'nocoll' replace AllGather with local copy; 'nodec' skip decoder."""
    import concourse.bacc as bacc
    import concourse.tile as tile
    import concourse.mybir as mybir
    from concourse import library_config

    f32 = mybir.dt.float32
    i16 = mybir.dt.int16
    dtt = mybir.dt.float16 if BF16 else mybir.dt.float32
    dttn = 2 if BF16 else 4   # bytes
    Alu = mybir.AluOpType
    Act = mybir.ActivationFunctionType

    S_lo, S_hi, Epad, T = meta["S_lo"], meta["S_hi"], meta["Epad"], meta["T"]
    tile_block = meta["tile_block"]
    tile_group = meta["tile_group"]
    t_start = meta["t_start"]
    t_stop = meta["t_stop"]

    # evac plan: group g -> mode at its last tile. 0: copy into aggT, 1: add
    B = meta["B"]
    first_half_of_block = {}
    evac_mode = {}
    for g in range(2 * NBLK):
        b = g % NBLK
        if B[g] == 0:
            continue
        if b not in first_half_of_block:
            first_half_of_block[b] = g
            evac_mode[g] = 0
        else:
            evac_mode[g] = 1

    nc = bacc.Bacc("TRN2", target_bir_lowering=False, debug=False,
                   enable_asserts=False, num_devices=1 if sim1 else C,
                   num_swdge_queues=NQ)

    # SWDGE completion sems are assigned round-robin (mod 8) over ALL SWDGE
    # DMA instructions in program order (tile_sem_assignment.next_sw_dma_idx),
    # and each sem is locked to one queue. queue = global_index % NQ (NQ
    # divides 8) keeps every sem slot on a single queue.
    _gq = [0]

    def next_queue():
        q = _gq[0] % NQ
        _gq[0] += 1
        return q

    def din(name, shape, dt=f32):
        return nc.dram_tensor(name, shape, dt, kind="ExternalInput").ap()

    h_oldT = din("h_oldT", [H, NPADC])
    idx_layer = din("idx_layer", [128, Epad // 16], i16)
    dstc_d = din("dstc", [128, T])
    dstc64_d = din("dstc64", [BLK, Epad], dtt)
    afT_d = din("afT", [9, Epad], dtt)
    cn = {k: din(k, [128, 128]) for k in [
        "Wf2", "c0_row", "Wrel0", "Wrel1", "Wrel2",
        "Wroot0", "Wroot1", "Wroot2", "brel0", "brel1", "brel2",
        "gam0", "gam1", "gam2", "bet0", "bet1", "bet2", "ident"]}
    for k in ("Wd1a", "Wd1b", "we2row", "ident16"):
        cn[k] = din(k, [128, 128], dtt)
    cn["wd2col"] = din("wd2col", [128, 1], dtt)
    cn["We1aug"] = din("We1aug", [9, 128], dtt)
    cn["Wd1caug"] = din("Wd1caug", [9, 128], dtt)
    cn["iota64"] = din("iota64", [128, BLK])
    cn["iotacol"] = din("iotacol", [BLK, 1])
    cn["epscol"] = din("epscol", [128, 1])
    cn["be2col"] = din("be2col", [128, 1])
    out_d = nc.dram_tensor("out", [1, Epad], f32, kind="ExternalOutput").ap()
    dbg_d = {}
    if "dbguv" in abl:
        dbg_d["U"] = nc.dram_tensor("dbg_U", [RT, H], dtt, kind="ExternalOutput").ap()
        dbg_d["V"] = nc.dram_tensor("dbg_V", [NPADC, H], dtt, kind="ExternalOutput").ap()
    if dbg:
        for l in range(L + 1):
            dbg_d[f"x{l}"] = nc.dram_tensor(
                f"dbg_x{l}", [NPADC, H], f32, kind="ExternalOutput").ap()
        dbg_d["ew"] = nc.dram_tensor("dbg_ew", [128, T], f32, kind="ExternalOutput").ap()
        dbg_d["agg0"] = nc.dram_tensor("dbg_agg0", [H, NPADC], f32, kind="ExternalOutput").ap()

    NT = NPADC // 128  # node tiles

    with tile.TileContext(nc) as tc:
        with tc.tile_pool(name="const", bufs=1) as cpool, \
             tc.tile_pool(name="dram", bufs=1, space="DRAM") as dpool:

            nc.gpsimd.load_library(library_config.mlp)

            ct = {}
            for k, d in cn.items():
                dt_ = dtt if k in ("We1aug", "we2row", "Wd1a", "Wd1b",
                                   "Wd1caug", "wd2col", "ident16") else f32
                t_ = cpool.tile(list(d.shape), dt_, tag=f"c_{k}", name=f"c_{k}")
                nc.sync.dma_start(t_[:], d[:])
                ct[k] = t_

            agin = [dpool.tile([NPADC, H], dtt, tag=f"agin{l}", name=f"agin{l}")
                    for l in range(L + 1)]
            tabs = [dpool.tile([RT, H], dtt, tag=f"tab{l}", name=f"tab{l}")
                    for l in range(L + 1)]
            # V = x3 @ Wd1b for own nodes (written at layer-3 export, read by
            # the decoder after the layer pools are freed)
            V_dram = dpool.tile([NPADC, H], dtt, tag="V_dram", name="V_dram")

            REP = 3 if "rep3" in abl else (2 if "rep2" in abl else 1)
            for _rep in range(REP):
                # ======== layer phase pools (freed before decoder) ========
                with tc.tile_pool(name="lpersist", bufs=1) as ppool, \
                     tc.tile_pool(name="work", bufs=2) as wpool, \
                     tc.tile_pool(name="gath", bufs=1) as gpool, \
                     tc.tile_pool(name="psum", bufs=4, space="PSUM") as pspool, \
                     tc.tile_pool(name="psum2", bufs=4, space="PSUM") as ps2pool:

                    dstc = ppool.tile([128, T], f32, tag="dstc")
                    nc.sync.dma_start(dstc[:], dstc_d[:])
                    ew = ppool.tile([128, T], f32, tag="ew")
                    x_own = ppool.tile([128, NT, H], f32, tag="x_own")
                    xT_own = ppool.tile([H, NPADC], f32, tag="xT_own")
                    aggT = ppool.tile([H, NPADC], f32, tag="aggT")

                    def export_tile(l, nt):
                        # l < L: export x_l rows (gather table for layer l).
                        # l == L: export U = x3 @ Wd1a instead (decoder table),
                        # and V = x3 @ Wd1b for own nodes via DRAM.
                        pst = ps2pool.tile([128, 128], f32, tag="ps_a")
                        nc.tensor.transpose(pst[:], x_own[:, nt, :], ct["ident"][:])
                        if l < L:
                            nc.vector.tensor_copy(
                                xT_own[:, nt * 128:(nt + 1) * 128], pst[:])
                            xrow = wpool.tile([128, 128], dtt, tag="w_xrow")
                            nc.scalar.copy(xrow[:], x_own[:, nt, :])
                            nc.sync.dma_start(
                                agin[l][nt * 128:(nt + 1) * 128, :], xrow[:])
                        else:
                            xTb = wpool.tile([128, 128], dtt, tag="w_xTb")
                            nc.scalar.copy(xTb[:], pst[:])
                            psU = ps2pool.tile([128, 128], f32, tag="ps_a")
                            nc.tensor.matmul(psU[:], xTb[:], ct["Wd1a"][:],
                                             start=True, stop=True)
                            urow = wpool.tile([128, 128], dtt, tag="w_xrow")
                            nc.scalar.copy(urow[:], psU[:])
                            nc.sync.dma_start(
                                agin[l][nt * 128:(nt + 1) * 128, :], urow[:])
                            psV = ps2pool.tile([128, 128], f32, tag="ps_a")
                            nc.tensor.matmul(psV[:], xTb[:], ct["Wd1b"][:],
                                             start=True, stop=True)
                            vrow = wpool.tile([128, 128], dtt, tag="w_vrow")
                            nc.scalar.copy(vrow[:], psV[:])
                            nc.sync.dma_start(
                                V_dram[nt * 128:(nt + 1) * 128, :], vrow[:])

                    def export_collective(l):
                        if sim1 or "nocoll" in abl:
                            nc.sync.dma_start(tabs[l][0:NPADC, :], agin[l][:])
                        else:
                            nc.gpsimd.collective_compute(
                                "AllGather", mybir.AluOpType.bypass,
                                ins=[agin[l].opt()], outs=[tabs[l].opt()],
                                replica_groups=[list(range(C))],
                            )

                    def node_transpose_and_export(l):
                        for nt in range(NT):
                            export_tile(l, nt)
                        export_collective(l)

                    # ===== Phase 1: x0 = relu(h_old @ Wf2 + c0)
                    for nt in range(NT):
                        hoT_t = wpool.tile([128, 128], f32, tag="w_hoT")
                        nc.sync.dma_start(hoT_t[:], h_oldT[:, nt * 128:(nt + 1) * 128])
                        ps = ps2pool.tile([128, 128], f32, tag="ps_a")
                        nc.tensor.matmul(ps[:], hoT_t[:], ct["Wf2"][:], start=True, stop=True)
                        tmp = wpool.tile([128, 128], f32, tag="w_init")
                        nc.vector.scalar_tensor_tensor(
                            tmp[:], ps[:], 1.0, ct["c0_row"][:], op0=Alu.mult, op1=Alu.add)
                        nc.vector.tensor_scalar_max(x_own[:, nt, :], tmp[:], 0.0)
                    node_transpose_and_export(0)
                    if dbg:
                        for nt in range(NT):
                            nc.sync.dma_start(dbg_d["x0"][nt * 128:(nt + 1) * 128, :],
                                              x_own[:, nt, :])

                    # ===== Phase 2: edge weights ew
                    AFC = 4096
                    for o in range(0, Epad, AFC):
                        n = min(AFC, Epad - o)
                        aft = wpool.tile([9, AFC], dtt, tag="w_aft")
                        nc.sync.dma_start(aft[:, :n], afT_d[:, o:o + n])
                        for tt in range(n // 128):
                            t0 = o // 128 + tt
                            pse = ps2pool.tile([128, 128], f32, tag="ps_a")
                            nc.tensor.matmul(pse[:], aft[:, tt * 128:(tt + 1) * 128],
                                             ct["We1aug"][:], start=True, stop=True)
                            h1 = wpool.tile([128, 128], dtt, tag="w_h1")
                            nc.scalar.activation(h1[:], pse[:], Act.Relu)
                            scr = wpool.tile([128, 128], f32, tag="w_scr")
                            nc.vector.scalar_tensor_tensor(
                                scr[:], h1[:], 1.0, ct["we2row"][:],
                                op0=Alu.mult, op1=Alu.mult,
                                accum_out=ew[:, t0:t0 + 1])
                    nc.scalar.activation(ew[:], ew[:], Act.Sigmoid, bias=ct["be2col"][:, 0:1])
                    if dbg:
                        nc.sync.dma_start(dbg_d["ew"][:], ew[:])

                    # ===== Phase 3: layers
                    def node_update(l, nt):
                        Wrel_t, Wroot_t = ct[f"Wrel{l}"], ct[f"Wroot{l}"]
                        brel_t, gam_t, bet_t = ct[f"brel{l}"], ct[f"gam{l}"], ct[f"bet{l}"]
                        sl = slice(nt * 128, (nt + 1) * 128)
                        psn = ps2pool.tile([128, 128], f32, tag="ps_a")
                        nc.tensor.matmul(psn[:], aggT[:, sl], Wrel_t[:],
                                         start=True, stop=False)
                        nc.tensor.matmul(psn[:], xT_own[:, sl], Wroot_t[:],
                                         start=False, stop=True)
                        hsb = wpool.tile([128, 128], f32, tag="w_h")
                        nc.vector.scalar_tensor_tensor(
                            hsb[:], psn[:], 1.0, brel_t[:], op0=Alu.mult, op1=Alu.add)
                        mu = wpool.tile([128, 1], f32, tag="w_mu")
                        nc.vector.tensor_reduce(mu[:], hsb[:],
                                                axis=mybir.AxisListType.X, op=Alu.add)
                        nc.scalar.mul(mu[:], mu[:], 1.0 / H)
                        cent = wpool.tile([128, 128], f32, tag="w_cent")
                        nc.vector.tensor_scalar(cent[:], hsb[:], mu[:], None,
                                                op0=Alu.subtract)
                        sq = wpool.tile([128, 128], f32, tag="w_sq")
                        vs = wpool.tile([128, 1], f32, tag="w_vs")
                        nc.scalar.activation(sq[:], cent[:], Act.Square, accum_out=vs[:])
                        std = wpool.tile([128, 1], f32, tag="w_std")
                        nc.scalar.activation(std[:], vs[:], Act.Sqrt,
                                             bias=ct["epscol"][:, 0:1], scale=1.0 / H)
                        rstd = wpool.tile([128, 1], f32, tag="w_rstd")
                        nc.vector.reciprocal(rstd[:], std[:])
                        t1 = wpool.tile([128, 128], f32, tag="w_t1")
                        nc.vector.scalar_tensor_tensor(
                            t1[:], cent[:], rstd[:], gam_t[:], op0=Alu.mult, op1=Alu.mult)
                        t2 = wpool.tile([128, 128], f32, tag="w_t2")
                        nc.vector.tensor_tensor(t2[:], t1[:], bet_t[:], op=Alu.add)
                        nc.vector.scalar_tensor_tensor(
                            x_own[:, nt, :], t2[:], 0.0, x_own[:, nt, :],
                            op0=Alu.max, op1=Alu.add)

                    # group-aligned gather chunks: PSUM accumulation chains
                    # never cross a chunk boundary
                    offs_b = np.zeros(2 * NBLK + 1, np.int64)
                    offs_b[1:] = np.cumsum(B)

                    def group_plan(g_lo, g_hi):
                        chunks = []
                        cur_o, cur_n = None, 0
                        for g in range(g_lo, g_hi):
                            bg = int(B[g])
                            assert bg <= GATHER_CHUNK
                            if cur_o is None:
                                cur_o, cur_n = int(offs_b[g]), 0
                            elif cur_n + bg > GATHER_CHUNK:
                                chunks.append((cur_o, cur_n))
                                cur_o, cur_n = int(offs_b[g]), 0
                            cur_n += bg
                        if cur_n:
                            chunks.append((cur_o, cur_n))
                        return chunks

                    gplan = group_plan(0, NBLK) + group_plan(NBLK, 2 * NBLK)

                    for l in range(L):
                        qi = 0
                        ps_blk = None
                        for (o, n) in gplan:
                            TT = n // 128
                            t0w = o // 128
                            gb = gpool.tile([128, GATHER_CHUNK // 128, H], dtt,
                                            tag="gbuf", bufs=GB_BUFS)
                            iw = gpool.tile([128, GATHER_CHUNK // 16], i16, tag="ibuf", bufs=6)
                            nc.sync.dma_start(iw[:, :n // 16],
                                              idx_layer[:, o // 16:(o + n) // 16])
                            src_ap = tabs[l][0:SPLIT, :] if o < S_lo else tabs[l][SPLIT:RT, :]
                            if "nogather" not in abl:
                                nc.gpsimd.dma_gather(
                                    gb[:, :TT, :], src_ap, iw[:, :n // 16],
                                    num_idxs=n, num_idxs_reg=n, elem_size=H,
                                    queue_num=next_queue(), single_packet=False)
                            else:
                                # same bytes, contiguous: isolates desc-gen cost
                                nc.sync.dma_start(
                                    gb[:, :TT, :],
                                    tabs[l][0:n, :].rearrange(
                                        "(p r) h -> p r h", p=128))
                            qi += 1
                            # one-hot * ew for the whole chunk: 2 wide DVE ops
                            eqw = gpool.tile([128, GATHER_CHUNK // 128, BLK], dtt,
                                             tag="eqw", bufs=3)
                            ohew = gpool.tile([128, GATHER_CHUNK // 128, BLK], dtt,
                                              tag="ohew", bufs=3)
                            dstc_b = dstc[:, t0w:t0w + TT].rearrange(
                                "p (t u) -> p t u", u=1).broadcast_to([128, TT, BLK])
                            ew_b = ew[:, t0w:t0w + TT].rearrange(
                                "p (t u) -> p t u", u=1).broadcast_to([128, TT, BLK])
                            iota_b = ct["iota64"][:].rearrange(
                                "p (u b) -> p u b", u=1).broadcast_to([128, TT, BLK])
                            nc.vector.tensor_tensor(eqw[:, :TT, :], dstc_b, iota_b,
                                                    op=Alu.is_equal)
                            nc.vector.tensor_tensor(ohew[:, :TT, :], eqw[:, :TT, :],
                                                    ew_b, op=Alu.mult)
                            for tt in range(TT):
                                t0 = t0w + tt
                                g = int(tile_group[t0])
                                b = int(tile_block[t0])
                                if t_start[t0]:
                                    ps_blk = pspool.tile([128, BLK], f32, tag="ps_sc")
                                nc.tensor.matmul(ps_blk[:], gb[:, tt, :], ohew[:, tt, :],
                                                 start=bool(t_start[t0]),
                                                 stop=bool(t_stop[t0]))
                                if t_stop[t0]:
                                    sl = aggT[:, b * BLK:(b + 1) * BLK]
                                    if evac_mode[g] == 0:
                                        nc.vector.tensor_copy(sl, ps_blk[:])
                                    else:
                                        nc.vector.tensor_tensor(sl, ps_blk[:], sl, op=Alu.add)
                                    # interleave node updates into the hi-pass:
                                    # pair (2nt, 2nt+1) is final once the hi
                                    # group of block 2nt+1 has been evacuated
                                    if g >= NBLK and b % 2 == 1:
                                        node_update(l, b // 2)
                                        export_tile(l + 1, b // 2)
                        if dbg and l == 0:
                            nc.sync.dma_start(dbg_d["agg0"][:, :], aggT[:, :])
                        export_collective(l + 1)
                        if dbg:
                            for nt in range(NT):
                                nc.sync.dma_start(
                                    dbg_d[f"x{l + 1}"][nt * 128:(nt + 1) * 128, :],
                                    x_own[:, nt, :])
                # lpersist + gath pools freed here

                # ========== Phase 4: decoder (reuses layer slot layout) =========
                # out[e] = wd2 . relu(U[src_e] + V[dst_e] + af_e @ Wd1c + bd1)
                # U[src]: HBM transposed gather from tabs[L] (concurrent-safe
                # across queues, unlike SBUF-source gathers).
                # V[dst]: slots are dst-block-grouped, so V comes from one-hot
                # matmuls against SBUF-resident V_blk; the one-hot [64, n] is
                # built on device from the streamed dstc64 row values.
                with tc.tile_pool(name="dec", bufs=1) as dcp, \
                     tc.tile_pool(name="psdec", bufs=3, space="PSUM") as psd_pool, \
                     tc.tile_pool(name="psout", bufs=4, space="PSUM") as pso_pool:
                    # V per 64-row dst block, base partition 0
                    V_blk = dcp.tile([BLK, NBLK, H], dtt, tag="V_blk")
                    nc.sync.dma_start(
                        V_blk[:, :, :],
                        V_dram[:].rearrange("(b r) h -> r b h", r=BLK))

                    if "dbguv" in abl:
                        nc.sync.dma_start(dbg_d["U"][:, :], tabs[L][:, :])
                        nc.sync.dma_start(dbg_d["V"][:, :], V_dram[:, :])

                    plan = (_chunks(S_lo, 0, GATHER_CHUNK_DEC)
                            + _chunks(S_hi, S_lo, GATHER_CHUNK_DEC))
                    if "nodec" in abl:
                        plan = []
                    for (o, n) in plan:
                        gu = dcp.tile([128, GATHER_CHUNK_DEC // 128, H], dtt,
                                      tag="gu", bufs=3)
                        iu = dcp.tile([128, GATHER_CHUNK_DEC // 16], i16, tag="ibuf2", bufs=3)
                        aft2 = dcp.tile([9, GATHER_CHUNK_DEC], dtt, tag="aft2", bufs=2)
                        d64 = dcp.tile([BLK, GATHER_CHUNK_DEC], dtt, tag="d64", bufs=2)
                        ohT = dcp.tile([BLK, GATHER_CHUNK_DEC], dtt, tag="ohT", bufs=2)
                        nc.sync.dma_start(iu[:, :n // 16],
                                          idx_layer[:, o // 16:(o + n) // 16])
                        nc.sync.dma_start(aft2[:, :n], afT_d[:, o:o + n])
                        nc.sync.dma_start(d64[:, :n], dstc64_d[:, o:o + n])
                        iotac_b = ct["iotacol"][:].broadcast_to([BLK, n])
                        nc.vector.tensor_tensor(ohT[:, :n], d64[:, :n], iotac_b,
                                                op=Alu.is_equal)
                        src_ap = tabs[L][0:SPLIT, :] if o < S_lo else tabs[L][SPLIT:RT, :]
                        if "nogather" not in abl and "nogu" not in abl:
                            # non-transposed gather [slot, H] (transposed
                            # gathers corrupt each other when overlapped)
                            nc.gpsimd.dma_gather(
                                gu[:, :n // 128, :], src_ap, iu[:, :n // 16],
                                num_idxs=n, num_idxs_reg=n, elem_size=H,
                                queue_num=0, single_packet=False)
                        else:
                            # same bytes, contiguous: isolates gather cost
                            nc.sync.dma_start(
                                gu[:, :n // 128, :],
                                tabs[L][0:n, :].rearrange("(p r) h -> p r h", p=128))
                        hmid = dcp.tile([128, GATHER_CHUNK_DEC], dtt, tag="hmid", bufs=2)
                        for (od, nd) in _chunks(n, 0, DEC_CHUNK):
                            psd = psd_pool.tile([128, DEC_CHUNK], f32, tag="ps_dec")
                            for tt in range(nd // 128):
                                t0 = (o + od) // 128 + tt
                                b = int(tile_block[t0])
                                sl = slice(od + tt * 128, od + (tt + 1) * 128)
                                sl2 = slice(tt * 128, (tt + 1) * 128)
                                nc.tensor.matmul(psd[:, sl2],
                                                 V_blk[0:BLK, b, :],
                                                 ohT[0:BLK, sl],
                                                 start=True, stop=False)
                                nc.tensor.matmul(psd[:, sl2], ct["Wd1caug"][:],
                                                 aft2[:, sl],
                                                 start=False, stop=False)
                                # += U[src]^T via identity (transposes gu tile)
                                nc.tensor.matmul(psd[:, sl2],
                                                 gu[:, (od + tt * 128) // 128, :],
                                                 ct["ident16"][:],
                                                 start=False, stop=True)
                            nc.scalar.activation(hmid[:, od:od + nd], psd[:, :nd],
                                                 Act.Relu)
                        orow = dcp.tile([1, GATHER_CHUNK_DEC], f32, tag="orow", bufs=2)
                        for (od, nd) in _chunks(n, 0, DEC_CHUNK):
                            pso = pso_pool.tile([1, DEC_CHUNK], f32, tag="ps_out")
                            nc.tensor.matmul(pso[:, :nd], ct["wd2col"][:],
                                             hmid[:, od:od + nd],
                                             start=True, stop=True)
                            nc.scalar.copy(orow[:, od:od + nd], pso[:, :nd])
                        nc.sync.dma_start(out_d[0:1, o:o + n], orow[:, :n])

    nc.compile()

    # Post-compile SWDGE queue assignment. Constraints learned the hard way:
    # (1) each DMASW completion sem (assigned positionally mod 8 over the
    #     final scheduled order) may only ever be incremented from ONE queue;
    # (2) concurrent SBUF-source gathers cross-contaminate their outputs, so
    #     all of them must share one queue (per-engine ring order serializes
    #     them); HBM-source gathers can spread across queues.
    # Assign queue PER SEM: 0 if any user is SBUF-source, else lane % NQ.
    from concourse.tile_scheduler import DMAInst
    from concourse import bass_isa as _bass_isa
    sem_users = {}
    for bb in nc.m.functions[0].blocks:
        for inst in bb.instructions:
            if (isinstance(inst, DMAInst)
                    and not isinstance(inst, _bass_isa.UserSyncedRemoteDMADescs)
                    and inst.engine == mybir.EngineType.Pool):
                upd = inst.sync_info.on_update[0]
                sem_users.setdefault(upd.ant_name, []).append(inst)
    for name, insts in sem_users.items():
        lane = int(name[len("DMASW"):].split("_")[0])
        if any(getattr(i, "sbuf_tokens_per_rank", 0) > 0 for i in insts):
            q = 0
        else:
            q = lane % NQ
        for i in insts:
            i.queue_num = q
    return nc


_CACHE = {}
DEBUG = False
TRACE = False
LAST = {}


def kernel(**inputs) -> np.ndarray:
    from concourse.bass_utils import run_bass_kernel_spmd

    meta, consts, per_core, bd2 = _prep(inputs)

    key = (meta["Epad"], tuple(meta["B"].tolist()), DEBUG)
    if key not in _CACHE:
        _CACHE[key] = _build(meta, dbg=DEBUG)
    nc = _CACHE[key]

    in_maps = []
    for c in range(C):
        pc = per_core[c]
        m = dict(consts)
        for k in ["h_oldT", "idx_layer", "dstc", "dstc64", "afT"]:
            m[k] = pc[k]
        in_maps.append(m)

    res = run_bass_kernel_spmd(nc, in_maps, core_ids=list(range(C)), trace=TRACE)
    LAST["exec_time_ns"] = res.exec_time_ns
    if DEBUG:
        LAST["res"] = res.results
        LAST["per_core"] = per_core
        LAST["meta"] = meta

    out = np.empty((E, 1), np.float32)
    for c in range(C):
        vals = res.results[c]["out"][0]      # [Epad], layer slot order
        inv = per_core[c]["inv"]
        good = inv >= 0
        out[inv[good], 0] = vals[good] + bd2[0]
    return out

